# revision 9
# baseline (speedup 1.0000x reference)
"""Trainium2 Bass kernel for nn_ChebyshevGraphConv (B=4, N=256, C=128, K=3).

Sharding: 8 cores = (batch b, node-half h). Core (b,h) owns node columns
jh = h*128:(h+1)*128 of the N x N edge grid for batch b:
  - msg side: msgs[b, jh, :] is complete locally (max over ALL i).
  - edge side: edge_out[b, :, jh, :] (pointwise in (i, j)).
  - phase D (laplacian/sigma/cheb + small per-batch compute) is done
    redundantly per core pair; each core writes out[b, jh, :].
No cross-core communication. One SPMD program with jh=0 baked in: cores
owning the second half receive block-permuted (node-relabeled) inputs,
un-permuted by the host on gather (the computation is equivariant under
node relabeling).

Device design:
  - X_T = edge_fts[b, :, jh, :]^T as [C, 32768] SBUF bf16, loaded with
    DMA-transpose straight from DRAM. All N^2-sized matmuls run in bf16
    (PE's fast fp32 path truncates the stationary operand to ~bf16
    anyway, so bf16 costs no extra accuracy); accumulation is fp32 in
    PSUM. All small per-batch math (laplacian, sigma, h, output head,
    layernorm) is plain fp32.
  - msg pipeline in A-layout [C, rows] 4-i quads (free=512).
  - edge pipeline in B-layout [j, c] per i with Horner combine
    out = s1*(s1*(X@2W2) + X@W1) + X@(W0-W2)  (s1 = cheb_lap col):
    ACT scale-copy + 2 DVE scalar_tensor_tensor. Edge work lags the
    quad loop by EDGE_LAG quads so the sigma chain is off the critical
    path.
  - sigma (largest singular value) via repeated squaring of M = L^T L
    with Frobenius normalization + Rayleigh quotient (the top spectrum
    of L is a near-continuum; plain power iteration cannot converge).
"""

import numpy as np
import ml_dtypes

B, N, C = 4, 256, 128
JH = 128
NQ = 64
EPS = 1e-5
LN_EPS = 1e-5
N_CORES = 8
N_SQUARINGS = 11
EDGE_LAG = 12

bf16 = ml_dtypes.bfloat16

_CACHE = {}

WF_NAMES = ["Wm1a", "Wm1b", "Wm2a", "Wm2b", "Wmg", "Wfe1a", "Wfe1b", "Wfe2",
            "Wo1", "Wo2", "Wo3a", "Wo3b", "nw0", "nw1", "nw2"]
WB_NAMES = ["Wme", "Wmlp1", "Wmlp2", "ew0", "ew1", "ew2"]
B_NAMES = ["b_m1", "b_me", "b_mg", "b_m2", "b_fe1", "b_fe2", "b_mlp1",
           "b_mlp2", "b_o1", "b_o2", "b_o3", "ln_g", "ln_b"]


def _build_nc():
    import concourse.bacc as bacc
    import concourse.mybir as mybir
    from concourse.tile import TileContext

    f32 = mybir.dt.float32
    bf = mybir.dt.bfloat16
    AF = mybir.ActivationFunctionType
    OP = mybir.AluOpType
    AX = mybir.AxisListType

    nc = bacc.Bacc("TRN2", target_bir_lowering=False)

    # ---------------- DRAM I/O ----------------
    xh_d = nc.dram_tensor("xh", [N * JH, C], bf, kind="ExternalInput")
    adjt_d = nc.dram_tensor("adjt", [N, N], f32, kind="ExternalInput")
    nodet_d = nc.dram_tensor("nodet", [C, N], f32, kind="ExternalInput")
    hidt_d = nc.dram_tensor("hidt", [C, N], f32, kind="ExternalInput")
    graph_d = nc.dram_tensor("graphc", [C, 1], f32, kind="ExternalInput")
    wd = {}
    for n in WF_NAMES:
        wd[n] = nc.dram_tensor(n, [C, C], f32, kind="ExternalInput")
    for n in WB_NAMES:
        wd[n] = nc.dram_tensor(n, [C, C], bf, kind="ExternalInput")
    bd = {n: nc.dram_tensor(n, [C, 1], f32, kind="ExternalInput") for n in B_NAMES}

    edge_o = nc.dram_tensor("edge_o", [N, JH, C], f32, kind="ExternalOutput")
    out_o = nc.dram_tensor("out_o", [C, JH], f32, kind="ExternalOutput")
    m2bounce = nc.dram_tensor("m2bounce", [N, C], bf)   # internal scratch

    id_cb = nc.inline_tensor(np.eye(C, dtype=bf16), "id_cb")
    eye256 = np.ascontiguousarray(
        np.eye(N, dtype=np.float32).reshape(2, 128, N).transpose(1, 0, 2))
    id_n = nc.inline_tensor(eye256, "id_n")
    ind4_c = nc.inline_tensor(
        np.kron(np.eye(4), np.ones((1, JH))).astype(bf16), "ind4")
    ones_col = nc.inline_tensor(np.ones((C, 1), np.float32), "ones_col")
    ones_row = nc.inline_tensor(np.ones((1, C), np.float32), "ones_row")

    with TileContext(nc) as tc:
        with tc.tile_pool(name="const", bufs=1) as cp, \
             tc.tile_pool(name="xt", bufs=1) as xp, \
             tc.tile_pool(name="msg", bufs=3) as mp, \
             tc.tile_pool(name="edge", bufs=4) as ep, \
             tc.tile_pool(name="small", bufs=1) as kp, \
             tc.tile_pool(name="pd", bufs=1) as dp, \
             tc.tile_pool(name="mps", bufs=4, space="PSUM") as mps, \
             tc.tile_pool(name="eps", bufs=2, space="PSUM") as eps_p, \
             tc.tile_pool(name="dps", bufs=2, space="PSUM") as dps:

            # ------------- loads (phase-D inputs first) -------------
            at_t = cp.tile([128, 2, N], f32, tag="at")
            nc.gpsimd.dma_start(
                at_t[:], adjt_d[:].rearrange("(t p) n -> p t n", p=128))
            idn_t = cp.tile([128, 2, N], f32, tag="idn")
            nc.gpsimd.dma_start(idn_t[:], id_n[:])
            onesc_t = cp.tile([C, 1], f32, tag="onesc")
            nc.gpsimd.dma_start(onesc_t[:], ones_col[:])
            onesr_t = cp.tile([1, C], f32, tag="onesr")
            nc.gpsimd.dma_start(onesr_t[:], ones_row[:])
            nodet_t = cp.tile([C, N], f32, tag="nodet")
            nc.gpsimd.dma_start(nodet_t[:], nodet_d[:])
            hidt_t = cp.tile([C, N], f32, tag="hidt")
            nc.gpsimd.dma_start(hidt_t[:], hidt_d[:])
            graph_t = cp.tile([C, 1], f32, tag="graph")
            nc.gpsimd.dma_start(graph_t[:], graph_d[:])
            w = {}
            for n in WB_NAMES:
                w[n] = cp.tile([C, C], bf, tag=n, name="w_" + n)
                nc.gpsimd.dma_start(w[n][:], wd[n][:])
            for n in WF_NAMES:
                w[n] = cp.tile([C, C], f32, tag=n, name="w_" + n)
                nc.gpsimd.dma_start(w[n][:], wd[n][:])
            b = {}
            for n in B_NAMES:
                b[n] = cp.tile([C, 1], f32, tag="t" + n, name="t" + n)
                nc.gpsimd.dma_start(b[n][:], bd[n][:])
            idb_t = cp.tile([C, C], bf, tag="idb")
            nc.gpsimd.dma_start(idb_t[:], id_cb[:])
            ind4_t = cp.tile([4, 4 * JH], bf, tag="ind4")
            nc.gpsimd.dma_start(ind4_t[:], ind4_c[:])

            # ============ PHASE D head: L, sigma, s1_T ============
            deg_t = kp.tile([128, 2], f32, tag="deg")
            nc.vector.tensor_reduce(deg_t[:], at_t[:], axis=AX.X, op=OP.add)
            sdeg_t = kp.tile([128, 2], f32, tag="sdeg")
            nc.scalar.activation(sdeg_t[:], deg_t[:], AF.Sqrt)
            dinv_t = kp.tile([128, 2], f32, tag="dinv")
            nc.vector.reciprocal(dinv_t[:], sdeg_t[:])
            dvr_ps = dps.tile([1, N], f32, tag="dps")
            for t in range(2):
                nc.tensor.matmul(dvr_ps[:, t * 128:(t + 1) * 128],
                                 dinv_t[:, t:t + 1],
                                 idn_t[:, t, t * 128:(t + 1) * 128],
                                 start=(t == 0), stop=(t == 1))
            dvr_t = kp.tile([1, N], f32, tag="dvr")
            nc.scalar.activation(dvr_t[:], dvr_ps[:], AF.Copy)
            lt_t = kp.tile([128, 2, N], f32, tag="lt")
            ln_t = kp.tile([128, 2, N], f32, tag="ln")
            for t in range(2):
                outer_ps = dps.tile([128, N], f32, tag="dps")
                nc.tensor.matmul(outer_ps[:], dvr_t[:, t * 128:(t + 1) * 128],
                                 dvr_t[:], start=True, stop=True)
                prod_t = dp.tile([128, N], f32, tag="prod")
                nc.vector.tensor_tensor(prod_t[:], at_t[:, t], outer_ps[:],
                                        OP.mult)
                nc.vector.tensor_tensor(lt_t[:, t], idn_t[:, t], prod_t[:],
                                        OP.subtract)
            for a in range(2):
                lnp = dps.tile([128, N], f32, tag="dps")
                for bb in range(2):
                    nc.tensor.transpose(lnp[:, bb * 128:(bb + 1) * 128],
                                        lt_t[:, bb, a * 128:(a + 1) * 128],
                                        idn_t[:, 0, 0:128])
                nc.scalar.activation(ln_t[:, a], lnp[:], AF.Copy)
            m0_t = kp.tile([128, 2, N], f32, tag="m0")
            mw_t = dp.tile([128, 2, N], f32, tag="mw", bufs=2)
            for t in range(2):
                mb_ps = dps.tile([128, N], f32, tag="dps")
                for a in range(2):
                    nc.tensor.matmul(mb_ps[:], ln_t[:, a, t * 128:(t + 1) * 128],
                                     ln_t[:, a], start=(a == 0), stop=(a == 1))
                nc.scalar.activation(m0_t[:, t], mb_ps[:], AF.Copy)
                nc.vector.tensor_copy(mw_t[:, t], mb_ps[:])
            for it in range(N_SQUARINGS):
                sqc_t = dp.tile([128, 2], f32, tag="sqc")
                for t in range(2):
                    scr = dp.tile([128, N], f32, tag="scr")
                    nc.scalar.activation(scr[:], mw_t[:, t], AF.Square,
                                         accum_out=sqc_t[:, t:t + 1])
                fro_ps = dps.tile([1, 1], f32, tag="dps")
                for t in range(2):
                    nc.tensor.matmul(fro_ps[:], sqc_t[:, t:t + 1], onesc_t[:],
                                     start=(t == 0), stop=(t == 1))
                fro_t = dp.tile([1, 1], f32, tag="fro")
                nc.scalar.activation(fro_t[:], fro_ps[:], AF.Sqrt)
                rfro_t = dp.tile([1, 1], f32, tag="rfro")
                nc.vector.reciprocal(rfro_t[:], fro_t[:])
                rfc_ps = dps.tile([C, 1], f32, tag="dps")
                nc.tensor.matmul(rfc_ps[:], onesr_t[:], rfro_t[:], start=True,
                                 stop=True)
                rfc_t = dp.tile([C, 1], f32, tag="rfc")
                nc.scalar.activation(rfc_t[:], rfc_ps[:], AF.Copy)
                mn_t = dp.tile([128, 2, N], f32, tag="mn", bufs=2)
                for t in range(2):
                    nc.vector.tensor_scalar(mn_t[:, t], mw_t[:, t], rfc_t[:],
                                            None, OP.mult)
                mw_t = dp.tile([128, 2, N], f32, tag="mw", bufs=2)
                for t in range(2):
                    sq_ps = dps.tile([128, N], f32, tag="dps")
                    for a in range(2):
                        nc.tensor.matmul(sq_ps[:],
                                         mn_t[:, a, t * 128:(t + 1) * 128],
                                         mn_t[:, a], start=(a == 0),
                                         stop=(a == 1))
                    nc.vector.tensor_copy(mw_t[:, t], sq_ps[:])
            v_t = kp.tile([128, 2], f32, tag="v")
            nc.vector.tensor_reduce(v_t[:], mw_t[:], axis=AX.X, op=OP.add)
            wv_t = kp.tile([128, 2], f32, tag="wv")
            for t in range(2):
                wv_ps = dps.tile([128, 1], f32, tag="dps")
                for a in range(2):
                    nc.tensor.matmul(wv_ps[:], m0_t[:, a, t * 128:(t + 1) * 128],
                                     v_t[:, a:a + 1], start=(a == 0),
                                     stop=(a == 1))
                nc.scalar.activation(wv_t[:, t:t + 1], wv_ps[:], AF.Copy)
            num_ps = dps.tile([1, 1], f32, tag="dps")
            den_ps = dps.tile([1, 1], f32, tag="dps")
            for t in range(2):
                nc.tensor.matmul(num_ps[:], v_t[:, t:t + 1], wv_t[:, t:t + 1],
                                 start=(t == 0), stop=(t == 1))
            for t in range(2):
                nc.tensor.matmul(den_ps[:], v_t[:, t:t + 1], v_t[:, t:t + 1],
                                 start=(t == 0), stop=(t == 1))
            den_s = dp.tile([1, 1], f32, tag="dens")
            nc.scalar.activation(den_s[:], den_ps[:], AF.Copy)
            rden_t = dp.tile([1, 1], f32, tag="rden")
            nc.vector.reciprocal(rden_t[:], den_s[:])
            lam_t = dp.tile([1, 1], f32, tag="lam")
            nc.vector.tensor_tensor(lam_t[:], num_ps[:], rden_t[:], OP.mult)
            sig_t = dp.tile([1, 1], f32, tag="sig")
            nc.scalar.activation(sig_t[:], lam_t[:], AF.Sqrt)
            sige_t = dp.tile([1, 1], f32, tag="sige")
            nc.vector.tensor_scalar(sige_t[:], sig_t[:], float(EPS), None,
                                    OP.add)
            rsig_t = dp.tile([1, 1], f32, tag="rsig")
            nc.vector.reciprocal(rsig_t[:], sige_t[:])
            c2_t = dp.tile([1, 1], f32, tag="c2")
            nc.vector.tensor_scalar(c2_t[:], rsig_t[:], 2.0, None, OP.mult)
            c2c_ps = dps.tile([C, 1], f32, tag="dps")
            nc.tensor.matmul(c2c_ps[:], onesr_t[:], c2_t[:], start=True,
                             stop=True)
            c2c_t = kp.tile([C, 1], f32, tag="c2c")
            nc.scalar.activation(c2c_t[:], c2c_ps[:], AF.Copy)
            s1t_t = kp.tile([128, 2, N], f32, tag="s1t")
            for t in range(2):
                nc.vector.scalar_tensor_tensor(s1t_t[:, t], lt_t[:, t],
                                               c2c_t[:], idn_t[:, t],
                                               OP.mult, OP.subtract)

            # ============ small precomputes: m1g, msg2, h, ewcat ============
            g_ps = dps.tile([C, 1], f32, tag="dps")
            nc.tensor.matmul(g_ps[:], w["Wmg"][:], graph_t[:], start=True,
                             stop=True)
            gcol_t = dp.tile([C, 1], f32, tag="gcol")
            nc.scalar.activation(gcol_t[:], g_ps[:], AF.Copy)
            gb_t = kp.tile([C, 1], f32, tag="gb")
            nc.vector.tensor_tensor(gb_t[:], gcol_t[:], b["b_m1"][:], OP.add)
            nc.vector.tensor_tensor(gb_t[:], gb_t[:], b["b_me"][:], OP.add)
            nc.vector.tensor_tensor(gb_t[:], gb_t[:], b["b_mg"][:], OP.add)

            m1_ps = dps.tile([C, JH], f32, tag="dps")
            nc.tensor.matmul(m1_ps[:], w["Wm1a"][:], nodet_t[:, 0:JH],
                             start=True, stop=False)
            nc.tensor.matmul(m1_ps[:], w["Wm1b"][:], hidt_t[:, 0:JH],
                             start=False, stop=True)
            m1g_t = kp.tile([C, 4 * JH], bf, tag="m1g")
            for q in range(4):
                nc.vector.tensor_scalar(m1g_t[:, q * JH:(q + 1) * JH],
                                        m1_ps[:], gb_t[:], None, OP.add)

            m2_ps = dps.tile([C, N], f32, tag="dps")
            nc.tensor.matmul(m2_ps[:], w["Wm2a"][:], nodet_t[:], start=True,
                             stop=False)
            nc.tensor.matmul(m2_ps[:], w["Wm2b"][:], hidt_t[:], start=False,
                             stop=True)
            msg2_t = kp.tile([C, N], f32, tag="msg2")
            nc.vector.tensor_scalar(msg2_t[:], m2_ps[:], b["b_m2"][:], None,
                                    OP.add)
            # transpose, bounce through DRAM to re-tile as [4, 64, C]
            m2n_t = dp.tile([128, 2, C], bf, tag="m2n")
            for t in range(2):
                tp_ps = dps.tile([128, C], f32, tag="dps")
                nc.tensor.transpose(tp_ps[:], msg2_t[:, t * 128:(t + 1) * 128],
                                    idn_t[:, 0, 0:128])
                nc.scalar.activation(m2n_t[:, t], tp_ps[:], AF.Copy)
            nc.gpsimd.dma_start(
                m2bounce[:].rearrange("(t p) c -> p t c", p=128), m2n_t[:])
            m2n4_t = kp.tile([4, NQ, C], bf, tag="m2n4")
            nc.gpsimd.dma_start(
                m2n4_t[:], m2bounce[:].rearrange("(g q) c -> q g c", q=4))

            h1_ps = dps.tile([C, N], f32, tag="dps")
            nc.tensor.matmul(h1_ps[:], w["Wfe1a"][:], nodet_t[:], start=True,
                             stop=False)
            nc.tensor.matmul(h1_ps[:], w["Wfe1b"][:], hidt_t[:], start=False,
                             stop=True)
            hh1_t = dp.tile([C, N], f32, tag="hh1")
            nc.scalar.activation(hh1_t[:], h1_ps[:], AF.Relu, bias=b["b_fe1"][:])
            h2_ps = dps.tile([C, N], f32, tag="dps")
            nc.tensor.matmul(h2_ps[:], w["Wfe2"][:], hh1_t[:], start=True,
                             stop=True)
            ht_t = kp.tile([C, N], f32, tag="ht")
            nc.vector.tensor_scalar(ht_t[:], h2_ps[:], b["b_fe2"][:], None,
                                    OP.add)
            hn_t = kp.tile([128, 2, C], f32, tag="hn")
            for t in range(2):
                tp_ps = dps.tile([128, C], f32, tag="dps")
                nc.tensor.transpose(tp_ps[:], ht_t[:, t * 128:(t + 1) * 128],
                                    idn_t[:, 0, 0:128])
                nc.scalar.activation(hn_t[:, t], tp_ps[:], AF.Copy)

            ewcat_t = cp.tile([C, 384], bf, tag="ewcat")
            nc.vector.tensor_tensor(ewcat_t[:, 0:128], w["ew0"][:],
                                    w["ew2"][:], OP.subtract)
            nc.vector.tensor_copy(ewcat_t[:, 128:256], w["ew1"][:])
            nc.vector.tensor_scalar(ewcat_t[:, 256:384], w["ew2"][:], 2.0,
                                    None, OP.mult)

            # ============ main loop ============
            xt_t = xp.tile([C, N * JH], bf, tag="xt")
            acc_t = kp.tile([C, 4 * JH], f32, tag="acc")
            nc.vector.memset(acc_t[:], -1e30)

            def do_edge(i):
                e_ps = eps_p.tile([JH, 384], f32, tag="eps", name="e_ps")
                nc.tensor.matmul(e_ps[:], xt_t[:, i * 128:(i + 1) * 128],
                                 ewcat_t[:], start=True, stop=True)
                s1c = s1t_t[:, 0, i:i + 1]
                u_t = ep.tile([JH, C], f32, tag="u", name="u_t")
                nc.scalar.activation(u_t[:], e_ps[:, 256:384], AF.Copy,
                                     scale=s1c)
                v2_t = ep.tile([JH, C], f32, tag="v2", name="v2_t")
                nc.vector.scalar_tensor_tensor(v2_t[:], u_t[:], s1c,
                                               e_ps[:, 128:256], OP.mult,
                                               OP.add)
                o_t = ep.tile([JH, C], f32, tag="o", name="o_t")
                nc.vector.scalar_tensor_tensor(o_t[:], v2_t[:], s1c,
                                               e_ps[:, 0:128], OP.mult,
                                               OP.add)
                nc.sync.dma_start(edge_o[i], o_t[:])

            for g in range(NQ):
                nc.sync.dma_start_transpose(
                    xt_t[:, g * 512:(g + 1) * 512],
                    xh_d[g * 512:(g + 1) * 512, :])

            for g in range(NQ):
                xq = xt_t[:, g * 512:(g + 1) * 512]
                pre_ps = mps.tile([C, 512], f32, tag="mps", name="pre_ps")
                nc.tensor.matmul(pre_ps[:], w["Wme"][:], xq, start=True,
                                 stop=False)
                nc.tensor.matmul(pre_ps[:], idb_t[:], m1g_t[:], start=False,
                                 stop=False)
                nc.tensor.matmul(pre_ps[:], m2n4_t[:, g, :], ind4_t[:],
                                 start=False, stop=True)
                h1q_t = mp.tile([C, 512], bf, tag="h1q", name="h1q_t")
                nc.scalar.activation(h1q_t[:], pre_ps[:], AF.Relu)
                p2_ps = mps.tile([C, 512], f32, tag="mps", name="p2_ps")
                nc.tensor.matmul(p2_ps[:], w["Wmlp1"][:], h1q_t[:], start=True,
                                 stop=True)
                h2q_t = mp.tile([C, 512], bf, tag="h2q", name="h2q_t")
                nc.scalar.activation(h2q_t[:], p2_ps[:], AF.Relu,
                                     bias=b["b_mlp1"][:])
                p3_ps = mps.tile([C, 512], f32, tag="mps", name="p3_ps")
                nc.tensor.matmul(p3_ps[:], w["Wmlp2"][:], h2q_t[:], start=True,
                                 stop=True)
                nc.vector.tensor_tensor(acc_t[:], acc_t[:], p3_ps[:], OP.max)

                if g >= EDGE_LAG:
                    gg = g - EDGE_LAG
                    for q in range(4):
                        do_edge(4 * gg + q)

            for gg in range(NQ - EDGE_LAG, NQ):
                for q in range(4):
                    do_edge(4 * gg + q)

            # ============ tail: msgs, cheb node side, out head, LN ============
            mx1_t = dp.tile([C, JH], f32, tag="mx1")
            nc.vector.tensor_tensor(mx1_t[:], acc_t[:, 0:128],
                                    acc_t[:, 128:256], OP.max)
            mx2_t = dp.tile([C, JH], f32, tag="mx2")
            nc.vector.tensor_tensor(mx2_t[:], acc_t[:, 256:384],
                                    acc_t[:, 384:512], OP.max)
            mx3_t = dp.tile([C, JH], f32, tag="mx3")
            nc.vector.tensor_tensor(mx3_t[:], mx1_t[:], mx2_t[:], OP.max)
            msgs_t = kp.tile([C, JH], f32, tag="msgs")
            nc.vector.tensor_scalar(msgs_t[:], mx3_t[:], b["b_mlp2"][:], None,
                                    OP.add)

            cn1_t = kp.tile([128, 2, C], f32, tag="cn1")
            for t in range(2):
                c_ps = dps.tile([128, C], f32, tag="dps")
                for a in range(2):
                    nc.tensor.matmul(c_ps[:], s1t_t[:, a, t * 128:(t + 1) * 128],
                                     hn_t[:, a], start=(a == 0), stop=(a == 1))
                nc.scalar.activation(cn1_t[:, t], c_ps[:], AF.Copy)
            c2_ps = dps.tile([128, C], f32, tag="dps")
            for a in range(2):
                nc.tensor.matmul(c2_ps[:], s1t_t[:, a, 0:128], cn1_t[:, a],
                                 start=(a == 0), stop=(a == 1))
            cn2_t = kp.tile([128, C], f32, tag="cn2")
            nc.vector.scalar_tensor_tensor(cn2_t[:], c2_ps[:], 2.0,
                                           hn_t[:, 0], OP.mult, OP.subtract)
            cn1T_ps = dps.tile([128, C], f32, tag="dps")
            nc.tensor.transpose(cn1T_ps[:], cn1_t[:, 0], idn_t[:, 0, 0:128])
            cn1T_t = dp.tile([C, JH], f32, tag="cn1T")
            nc.scalar.activation(cn1T_t[:], cn1T_ps[:], AF.Copy)
            cn2T_ps = dps.tile([128, C], f32, tag="dps")
            nc.tensor.transpose(cn2T_ps[:], cn2_t[:], idn_t[:, 0, 0:128])
            cn2T_t = dp.tile([C, JH], f32, tag="cn2T")
            nc.scalar.activation(cn2T_t[:], cn2T_ps[:], AF.Copy)

            no_t = dp.tile([C, JH], f32, tag="no")
            tmp_t = dp.tile([C, JH], f32, tag="tmp")
            for k, cnk in enumerate([ht_t[:, 0:JH], cn1T_t[:], cn2T_t[:]]):
                pj_ps = dps.tile([C, JH], f32, tag="dps")
                nc.tensor.matmul(pj_ps[:], w["nw%d" % k][:], msgs_t[:],
                                 start=True, stop=True)
                if k == 0:
                    nc.vector.tensor_tensor(no_t[:], cnk, pj_ps[:], OP.mult)
                else:
                    nc.vector.tensor_tensor(tmp_t[:], cnk, pj_ps[:], OP.mult)
                    nc.vector.tensor_tensor(no_t[:], no_t[:], tmp_t[:], OP.add)

            oh_ps = dps.tile([C, JH], f32, tag="dps")
            nc.tensor.matmul(oh_ps[:], w["Wo1"][:], msgs_t[:], start=True,
                             stop=False)
            nc.tensor.matmul(oh_ps[:], w["Wo2"][:], no_t[:], start=False,
                             stop=False)
            nc.tensor.matmul(oh_ps[:], w["Wo3a"][:], nodet_t[:, 0:JH],
                             start=False, stop=False)
            nc.tensor.matmul(oh_ps[:], w["Wo3b"][:], hidt_t[:, 0:JH],
                             start=False, stop=True)
            bo_t = dp.tile([C, 1], f32, tag="bo")
            nc.vector.tensor_tensor(bo_t[:], b["b_o1"][:], b["b_o2"][:], OP.add)
            nc.vector.tensor_tensor(bo_t[:], bo_t[:], b["b_o3"][:], OP.add)
            opre_t = kp.tile([C, JH], f32, tag="opre")
            nc.vector.tensor_scalar(opre_t[:], oh_ps[:], bo_t[:], None, OP.add)

            sq_t = dp.tile([C, JH], f32, tag="sq")
            nc.scalar.activation(sq_t[:], opre_t[:], AF.Square)
            mu_ps = dps.tile([1, JH], f32, tag="dps")
            nc.tensor.matmul(mu_ps[:], onesc_t[:], opre_t[:], start=True,
                             stop=True)
            s2_ps = dps.tile([1, JH], f32, tag="dps")
            nc.tensor.matmul(s2_ps[:], onesc_t[:], sq_t[:], start=True,
                             stop=True)
            mu_t = dp.tile([1, JH], f32, tag="mu")
            nc.vector.tensor_scalar(mu_t[:], mu_ps[:], 1.0 / C, None, OP.mult)
            musq_t = dp.tile([1, JH], f32, tag="musq")
            nc.scalar.activation(musq_t[:], mu_t[:], AF.Square)
            var_t = dp.tile([1, JH], f32, tag="var")
            nc.vector.tensor_scalar(var_t[:], s2_ps[:], 1.0 / C, None, OP.mult)
            nc.vector.tensor_tensor(var_t[:], var_t[:], musq_t[:], OP.subtract)
            nc.vector.tensor_scalar(var_t[:], var_t[:], float(LN_EPS), None,
                                    OP.add)
            sd_t = dp.tile([1, JH], f32, tag="sd")
            nc.scalar.activation(sd_t[:], var_t[:], AF.Sqrt)
            r_t = dp.tile([1, JH], f32, tag="r")
            nc.vector.reciprocal(r_t[:], sd_t[:])
            u2_t = dp.tile([1, JH], f32, tag="u2")
            nc.vector.tensor_tensor(u2_t[:], mu_t[:], r_t[:], OP.mult)
            one1 = dp.tile([1, 1], f32, tag="one1")
            nc.vector.memset(one1[:], 1.0)
            rc_ps = dps.tile([JH, 1], f32, tag="dps")
            nc.tensor.matmul(rc_ps[:], r_t[:], one1[:], start=True, stop=True)
            rc_t = dp.tile([JH, 1], f32, tag="rc")
            nc.scalar.activation(rc_t[:], rc_ps[:], AF.Copy)
            uc_ps = dps.tile([JH, 1], f32, tag="dps")
            nc.tensor.matmul(uc_ps[:], u2_t[:], one1[:], start=True, stop=True)
            uc_t = dp.tile([JH, 1], f32, tag="uc")
            nc.scalar.activation(uc_t[:], uc_ps[:], AF.Copy)
            on_ps = dps.tile([JH, C], f32, tag="dps")
            nc.tensor.transpose(on_ps[:], opre_t[:], idn_t[:, 0, 0:128])
            yn_t = dp.tile([JH, C], f32, tag="yn")
            nc.vector.tensor_scalar(yn_t[:], on_ps[:], rc_t[:], uc_t[:],
                                    OP.mult, OP.subtract)
            yt_ps = dps.tile([C, JH], f32, tag="dps")
            nc.tensor.transpose(yt_ps[:], yn_t[:], idn_t[:, 0, 0:128])
            yf_t = dp.tile([C, JH], f32, tag="yf")
            nc.vector.tensor_scalar(yf_t[:], yt_ps[:], b["ln_g"][:],
                                    b["ln_b"][:], OP.mult, OP.add)
            nc.sync.dma_start(out_o[:], yf_t[:])

    nc.compile()
    return nc


def _get_nc():
    if "nc" not in _CACHE:
        _CACHE["nc"] = _build_nc()
    return _CACHE["nc"]


def make_in_maps(inputs):
    """Build the 8 per-core input dicts (host-side sharding)."""
    node_fts = np.asarray(inputs["node_fts"], np.float32)
    edge_fts = np.asarray(inputs["edge_fts"], np.float32)
    graph_fts = np.asarray(inputs["graph_fts"], np.float32)
    hidden = np.asarray(inputs["hidden"], np.float32)
    adj = np.asarray(inputs["adj_matrix"], np.float32)

    wmap_c = {}
    wmap_c["Wm1a"] = inputs["W_m1"][0:C]
    wmap_c["Wm1b"] = inputs["W_m1"][C:2 * C]
    wmap_c["Wm2a"] = inputs["W_m2"][0:C]
    wmap_c["Wm2b"] = inputs["W_m2"][C:2 * C]
    wmap_c["Wme"] = inputs["W_me"]
    wmap_c["Wmg"] = inputs["W_mg"]
    wmap_c["Wfe1a"] = inputs["W_fe1"][0:C]
    wmap_c["Wfe1b"] = inputs["W_fe1"][C:2 * C]
    wmap_c["Wfe2"] = inputs["W_fe2"]
    wmap_c["Wmlp1"] = inputs["W_mlp1"]
    wmap_c["Wmlp2"] = inputs["W_mlp2"]
    wmap_c["Wo1"] = inputs["W_o1"]
    wmap_c["Wo2"] = inputs["W_o2"]
    wmap_c["Wo3a"] = inputs["W_o3"][0:C]
    wmap_c["Wo3b"] = inputs["W_o3"][C:2 * C]
    for k in range(3):
        wmap_c["nw%d" % k] = inputs["node_weights"][k]
        wmap_c["ew%d" % k] = inputs["edge_weights"][k]
    wmap = {}
    for n, a in wmap_c.items():
        a = np.ascontiguousarray(np.asarray(a, np.float32))
        if n in WB_NAMES:
            wmap[n] = a.astype(bf16)
        else:
            wmap[n] = a
    bmap = {
        "b_m1": inputs["b_m1"], "b_me": inputs["b_me"], "b_mg": inputs["b_mg"],
        "b_m2": inputs["b_m2"], "b_fe1": inputs["b_fe1"],
        "b_fe2": inputs["b_fe2"], "b_mlp1": inputs["b_mlp1"],
        "b_mlp2": inputs["b_mlp2"], "b_o1": inputs["b_o1"],
        "b_o2": inputs["b_o2"], "b_o3": inputs["b_o3"],
        "ln_g": inputs["ln_g"], "ln_b": inputs["ln_b"],
    }
    bmap = {n: np.ascontiguousarray(
        np.asarray(a, np.float32).reshape(C, 1)) for n, a in bmap.items()}

    in_maps = []
    perms = [np.arange(N), np.concatenate([np.arange(128, 256),
                                           np.arange(0, 128)])]
    for core in range(N_CORES):
        bidx, h = core // 2, core % 2
        perm = perms[h]
        xs = edge_fts[bidx][perm][:, perm[:JH], :]       # [N, JH, C]
        a2 = adj[bidx][np.ix_(perm, perm)]
        m = {
            "xh": np.ascontiguousarray(xs.reshape(N * JH, C).astype(bf16)),
            "adjt": np.ascontiguousarray(a2.T),
            "nodet": np.ascontiguousarray(node_fts[bidx][perm].T),
            "hidt": np.ascontiguousarray(hidden[bidx][perm].T),
            "graphc": np.ascontiguousarray(graph_fts[bidx].reshape(C, 1)),
        }
        m.update(wmap)
        m.update(bmap)
        in_maps.append(m)
    return in_maps


def gather(results):
    """Assemble full outputs from the 8 per-core result dicts."""
    out = np.zeros((B, N, C), np.float32)
    edge_out = np.zeros((B, N, N, C), np.float32)
    perms = [np.arange(N), np.concatenate([np.arange(128, 256),
                                           np.arange(0, 128)])]
    for core in range(N_CORES):
        bidx, h = core // 2, core % 2
        perm = perms[h]
        jcols = perm[:JH]
        edge_out[bidx][np.ix_(perm, jcols)] = results[core]["edge_o"]
        out[bidx][jcols] = results[core]["out_o"].T
    return out, edge_out


def kernel(**inputs):
    from concourse.bass_utils import run_bass_kernel_spmd

    nc = _get_nc()
    in_maps = make_in_maps(inputs)
    res = run_bass_kernel_spmd(nc, in_maps, list(range(N_CORES)))
    return gather(res.results)


# revision 10
# speedup vs baseline: 1.1387x; 1.1387x over previous
"""Trainium2 Bass kernel for nn_ChebyshevGraphConv (B=4, N=256, C=128, K=3).

Sharding: 8 cores = (batch b, node-half h). Core (b,h) owns node columns
jh = h*128:(h+1)*128 of the N x N edge grid for batch b:
  - msg side: msgs[b, jh, :] is complete locally (max over ALL i).
  - edge side: edge_out[b, :, jh, :] (pointwise in (i, j)).
  - phase D (laplacian/sigma/cheb + small per-batch compute) is done
    redundantly per core pair; each core writes out[b, jh, :].
No cross-core communication. One SPMD program with jh=0 baked in: cores
owning the second half receive block-permuted (node-relabeled) inputs,
un-permuted by the host on gather (the computation is equivariant under
node relabeling).

Device design:
  - X_T = edge_fts[b, :, jh, :]^T as [C, 32768] SBUF bf16, loaded with
    DMA-transpose straight from DRAM. All N^2-sized matmuls run in bf16
    (PE's fast fp32 path truncates the stationary operand to ~bf16
    anyway, so bf16 costs no extra accuracy); accumulation is fp32 in
    PSUM. All small per-batch math (laplacian, sigma, h, output head,
    layernorm) is plain fp32.
  - msg pipeline in A-layout [C, rows] 4-i quads (free=512).
  - edge pipeline in B-layout [j, c] per i with Horner combine
    out = s1*(s1*(X@2W2) + X@W1) + X@(W0-W2)  (s1 = cheb_lap col):
    ACT scale-copy + 2 DVE scalar_tensor_tensor. Edge work lags the
    quad loop by EDGE_LAG quads so the sigma chain is off the critical
    path.
  - sigma (largest singular value) via repeated squaring of M = L^T L
    with Frobenius normalization + Rayleigh quotient (the top spectrum
    of L is a near-continuum; plain power iteration cannot converge).
"""

import numpy as np
import ml_dtypes

B, N, C = 4, 256, 128
JH = 128
NQ = 64
EPS = 1e-5
LN_EPS = 1e-5
N_CORES = 8
N_SQUARINGS = 11
EDGE_LAG = 12

bf16 = ml_dtypes.bfloat16

_CACHE = {}

WF_NAMES = ["Wm1a", "Wm1b", "Wm2a", "Wm2b", "Wmg", "Wfe1a", "Wfe1b", "Wfe2",
            "Wo1", "Wo2", "Wo3a", "Wo3b", "nw0", "nw1", "nw2"]
WB_NAMES = ["Wme", "Wmlp1", "Wmlp2", "ew0", "ew1", "ew2"]
B_NAMES = ["b_m1", "b_me", "b_mg", "b_m2", "b_fe1", "b_fe2", "b_mlp1",
           "b_mlp2", "b_o1", "b_o2", "b_o3", "ln_g", "ln_b"]


def _build_nc():
    import concourse.bacc as bacc
    import concourse.mybir as mybir
    from concourse.tile import TileContext

    f32 = mybir.dt.float32
    bf = mybir.dt.bfloat16
    AF = mybir.ActivationFunctionType
    OP = mybir.AluOpType
    AX = mybir.AxisListType

    nc = bacc.Bacc("TRN2", target_bir_lowering=False)

    # ---------------- DRAM I/O ----------------
    xh_d = nc.dram_tensor("xh", [N * JH, C], bf, kind="ExternalInput")
    adjt_d = nc.dram_tensor("adjt", [N, N], f32, kind="ExternalInput")
    nodet_d = nc.dram_tensor("nodet", [C, N], f32, kind="ExternalInput")
    hidt_d = nc.dram_tensor("hidt", [C, N], f32, kind="ExternalInput")
    graph_d = nc.dram_tensor("graphc", [C, 1], f32, kind="ExternalInput")
    wd = {}
    for n in WF_NAMES:
        wd[n] = nc.dram_tensor(n, [C, C], f32, kind="ExternalInput")
    for n in WB_NAMES:
        wd[n] = nc.dram_tensor(n, [C, C], bf, kind="ExternalInput")
    bd = {n: nc.dram_tensor(n, [C, 1], f32, kind="ExternalInput") for n in B_NAMES}

    edge_o = nc.dram_tensor("edge_o", [N, JH, C], f32, kind="ExternalOutput")
    out_o = nc.dram_tensor("out_o", [C, JH], f32, kind="ExternalOutput")
    m2bounce = nc.dram_tensor("m2bounce", [N, C], bf)   # internal scratch

    id_cb = nc.inline_tensor(np.eye(C, dtype=bf16), "id_cb")
    eye256 = np.ascontiguousarray(
        np.eye(N, dtype=np.float32).reshape(2, 128, N).transpose(1, 0, 2))
    id_n = nc.inline_tensor(eye256, "id_n")
    ind4_c = nc.inline_tensor(
        np.kron(np.eye(4), np.ones((1, JH))).astype(bf16), "ind4")
    ones_col = nc.inline_tensor(np.ones((C, 1), np.float32), "ones_col")
    ones_row = nc.inline_tensor(np.ones((1, C), np.float32), "ones_row")

    with TileContext(nc) as tc:
        with tc.tile_pool(name="const", bufs=1) as cp, \
             tc.tile_pool(name="xt", bufs=1) as xp, \
             tc.tile_pool(name="msg", bufs=3) as mp, \
             tc.tile_pool(name="edge", bufs=4) as ep, \
             tc.tile_pool(name="small", bufs=1) as kp, \
             tc.tile_pool(name="pd", bufs=1) as dp, \
             tc.tile_pool(name="mps", bufs=3, space="PSUM") as mps, \
             tc.tile_pool(name="eps", bufs=3, space="PSUM") as eps_p, \
             tc.tile_pool(name="dps", bufs=2, space="PSUM") as dps:

            # ------------- loads (phase-D inputs first) -------------
            at_t = cp.tile([128, 2, N], f32, tag="at")
            nc.gpsimd.dma_start(
                at_t[:], adjt_d[:].rearrange("(t p) n -> p t n", p=128))
            idn_t = cp.tile([128, 2, N], f32, tag="idn")
            nc.gpsimd.dma_start(idn_t[:], id_n[:])
            onesc_t = cp.tile([C, 1], f32, tag="onesc")
            nc.gpsimd.dma_start(onesc_t[:], ones_col[:])
            onesr_t = cp.tile([1, C], f32, tag="onesr")
            nc.gpsimd.dma_start(onesr_t[:], ones_row[:])
            nodet_t = cp.tile([C, N], f32, tag="nodet")
            nc.gpsimd.dma_start(nodet_t[:], nodet_d[:])
            hidt_t = cp.tile([C, N], f32, tag="hidt")
            nc.gpsimd.dma_start(hidt_t[:], hidt_d[:])
            graph_t = cp.tile([C, 1], f32, tag="graph")
            nc.gpsimd.dma_start(graph_t[:], graph_d[:])
            w = {}
            for n in WB_NAMES:
                w[n] = cp.tile([C, C], bf, tag=n, name="w_" + n)
                nc.gpsimd.dma_start(w[n][:], wd[n][:])
            for n in WF_NAMES:
                w[n] = cp.tile([C, C], f32, tag=n, name="w_" + n)
                nc.gpsimd.dma_start(w[n][:], wd[n][:])
            b = {}
            for n in B_NAMES:
                b[n] = cp.tile([C, 1], f32, tag="t" + n, name="t" + n)
                nc.gpsimd.dma_start(b[n][:], bd[n][:])
            idb_t = cp.tile([C, C], bf, tag="idb")
            nc.gpsimd.dma_start(idb_t[:], id_cb[:])
            ind4_t = cp.tile([4, 4 * JH], bf, tag="ind4")
            nc.gpsimd.dma_start(ind4_t[:], ind4_c[:])

            # ============ PHASE D head: L, sigma, s1_T ============
            deg_t = kp.tile([128, 2], f32, tag="deg")
            nc.vector.tensor_reduce(deg_t[:], at_t[:], axis=AX.X, op=OP.add)
            sdeg_t = kp.tile([128, 2], f32, tag="sdeg")
            nc.scalar.activation(sdeg_t[:], deg_t[:], AF.Sqrt)
            dinv_t = kp.tile([128, 2], f32, tag="dinv")
            nc.vector.reciprocal(dinv_t[:], sdeg_t[:])
            dvr_ps = dps.tile([1, N], f32, tag="dps")
            for t in range(2):
                nc.tensor.matmul(dvr_ps[:, t * 128:(t + 1) * 128],
                                 dinv_t[:, t:t + 1],
                                 idn_t[:, t, t * 128:(t + 1) * 128],
                                 start=(t == 0), stop=(t == 1))
            dvr_t = kp.tile([1, N], f32, tag="dvr")
            nc.scalar.activation(dvr_t[:], dvr_ps[:], AF.Copy)
            lt_t = kp.tile([128, 2, N], f32, tag="lt")
            ln_t = kp.tile([128, 2, N], f32, tag="ln")
            for t in range(2):
                outer_ps = dps.tile([128, N], f32, tag="dps")
                nc.tensor.matmul(outer_ps[:], dvr_t[:, t * 128:(t + 1) * 128],
                                 dvr_t[:], start=True, stop=True)
                prod_t = dp.tile([128, N], f32, tag="prod")
                nc.vector.tensor_tensor(prod_t[:], at_t[:, t], outer_ps[:],
                                        OP.mult)
                nc.vector.tensor_tensor(lt_t[:, t], idn_t[:, t], prod_t[:],
                                        OP.subtract)
            for a in range(2):
                lnp = dps.tile([128, N], f32, tag="dps")
                for bb in range(2):
                    nc.tensor.transpose(lnp[:, bb * 128:(bb + 1) * 128],
                                        lt_t[:, bb, a * 128:(a + 1) * 128],
                                        idn_t[:, 0, 0:128])
                nc.scalar.activation(ln_t[:, a], lnp[:], AF.Copy)
            m0_t = kp.tile([128, 2, N], f32, tag="m0")
            mw_t = dp.tile([128, 2, N], f32, tag="mw", bufs=2)
            for t in range(2):
                mb_ps = dps.tile([128, N], f32, tag="dps")
                for a in range(2):
                    nc.tensor.matmul(mb_ps[:], ln_t[:, a, t * 128:(t + 1) * 128],
                                     ln_t[:, a], start=(a == 0), stop=(a == 1))
                nc.scalar.activation(m0_t[:, t], mb_ps[:], AF.Copy)
                nc.vector.tensor_copy(mw_t[:, t], mb_ps[:])
            for it in range(N_SQUARINGS):
                sqc_t = dp.tile([128, 2], f32, tag="sqc")
                for t in range(2):
                    scr = dp.tile([128, N], f32, tag="scr")
                    nc.scalar.activation(scr[:], mw_t[:, t], AF.Square,
                                         accum_out=sqc_t[:, t:t + 1])
                fro_ps = dps.tile([1, 1], f32, tag="dps")
                for t in range(2):
                    nc.tensor.matmul(fro_ps[:], sqc_t[:, t:t + 1], onesc_t[:],
                                     start=(t == 0), stop=(t == 1))
                fro_t = dp.tile([1, 1], f32, tag="fro")
                nc.scalar.activation(fro_t[:], fro_ps[:], AF.Sqrt)
                rfro_t = dp.tile([1, 1], f32, tag="rfro")
                nc.vector.reciprocal(rfro_t[:], fro_t[:])
                rfc_ps = dps.tile([C, 1], f32, tag="dps")
                nc.tensor.matmul(rfc_ps[:], onesr_t[:], rfro_t[:], start=True,
                                 stop=True)
                rfc_t = dp.tile([C, 1], f32, tag="rfc")
                nc.scalar.activation(rfc_t[:], rfc_ps[:], AF.Copy)
                mn_t = dp.tile([128, 2, N], f32, tag="mn", bufs=2)
                for t in range(2):
                    nc.vector.tensor_scalar(mn_t[:, t], mw_t[:, t], rfc_t[:],
                                            None, OP.mult)
                mw_t = dp.tile([128, 2, N], f32, tag="mw", bufs=2)
                for t in range(2):
                    sq_ps = dps.tile([128, N], f32, tag="dps")
                    for a in range(2):
                        nc.tensor.matmul(sq_ps[:],
                                         mn_t[:, a, t * 128:(t + 1) * 128],
                                         mn_t[:, a], start=(a == 0),
                                         stop=(a == 1))
                    nc.vector.tensor_copy(mw_t[:, t], sq_ps[:])
            v_t = kp.tile([128, 2], f32, tag="v")
            nc.vector.tensor_reduce(v_t[:], mw_t[:], axis=AX.X, op=OP.add)
            wv_t = kp.tile([128, 2], f32, tag="wv")
            for t in range(2):
                wv_ps = dps.tile([128, 1], f32, tag="dps")
                for a in range(2):
                    nc.tensor.matmul(wv_ps[:], m0_t[:, a, t * 128:(t + 1) * 128],
                                     v_t[:, a:a + 1], start=(a == 0),
                                     stop=(a == 1))
                nc.scalar.activation(wv_t[:, t:t + 1], wv_ps[:], AF.Copy)
            num_ps = dps.tile([1, 1], f32, tag="dps")
            den_ps = dps.tile([1, 1], f32, tag="dps")
            for t in range(2):
                nc.tensor.matmul(num_ps[:], v_t[:, t:t + 1], wv_t[:, t:t + 1],
                                 start=(t == 0), stop=(t == 1))
            for t in range(2):
                nc.tensor.matmul(den_ps[:], v_t[:, t:t + 1], v_t[:, t:t + 1],
                                 start=(t == 0), stop=(t == 1))
            den_s = dp.tile([1, 1], f32, tag="dens")
            nc.scalar.activation(den_s[:], den_ps[:], AF.Copy)
            rden_t = dp.tile([1, 1], f32, tag="rden")
            nc.vector.reciprocal(rden_t[:], den_s[:])
            lam_t = dp.tile([1, 1], f32, tag="lam")
            nc.vector.tensor_tensor(lam_t[:], num_ps[:], rden_t[:], OP.mult)
            sig_t = dp.tile([1, 1], f32, tag="sig")
            nc.scalar.activation(sig_t[:], lam_t[:], AF.Sqrt)
            sige_t = dp.tile([1, 1], f32, tag="sige")
            nc.vector.tensor_scalar(sige_t[:], sig_t[:], float(EPS), None,
                                    OP.add)
            rsig_t = dp.tile([1, 1], f32, tag="rsig")
            nc.vector.reciprocal(rsig_t[:], sige_t[:])
            c2_t = dp.tile([1, 1], f32, tag="c2")
            nc.vector.tensor_scalar(c2_t[:], rsig_t[:], 2.0, None, OP.mult)
            c2c_ps = dps.tile([C, 1], f32, tag="dps")
            nc.tensor.matmul(c2c_ps[:], onesr_t[:], c2_t[:], start=True,
                             stop=True)
            c2c_t = kp.tile([C, 1], f32, tag="c2c")
            nc.scalar.activation(c2c_t[:], c2c_ps[:], AF.Copy)
            s1t_t = kp.tile([128, 2, N], f32, tag="s1t")
            for t in range(2):
                nc.vector.scalar_tensor_tensor(s1t_t[:, t], lt_t[:, t],
                                               c2c_t[:], idn_t[:, t],
                                               OP.mult, OP.subtract)

            # ============ small precomputes: m1g, msg2, h, ewcat ============
            g_ps = dps.tile([C, 1], f32, tag="dps")
            nc.tensor.matmul(g_ps[:], w["Wmg"][:], graph_t[:], start=True,
                             stop=True)
            gcol_t = dp.tile([C, 1], f32, tag="gcol")
            nc.scalar.activation(gcol_t[:], g_ps[:], AF.Copy)
            gb_t = kp.tile([C, 1], f32, tag="gb")
            nc.vector.tensor_tensor(gb_t[:], gcol_t[:], b["b_m1"][:], OP.add)
            nc.vector.tensor_tensor(gb_t[:], gb_t[:], b["b_me"][:], OP.add)
            nc.vector.tensor_tensor(gb_t[:], gb_t[:], b["b_mg"][:], OP.add)

            m1_ps = dps.tile([C, JH], f32, tag="dps")
            nc.tensor.matmul(m1_ps[:], w["Wm1a"][:], nodet_t[:, 0:JH],
                             start=True, stop=False)
            nc.tensor.matmul(m1_ps[:], w["Wm1b"][:], hidt_t[:, 0:JH],
                             start=False, stop=True)
            m1g_t = kp.tile([C, 4 * JH], bf, tag="m1g")
            for q in range(4):
                nc.vector.tensor_scalar(m1g_t[:, q * JH:(q + 1) * JH],
                                        m1_ps[:], gb_t[:], None, OP.add)

            m2_ps = dps.tile([C, N], f32, tag="dps")
            nc.tensor.matmul(m2_ps[:], w["Wm2a"][:], nodet_t[:], start=True,
                             stop=False)
            nc.tensor.matmul(m2_ps[:], w["Wm2b"][:], hidt_t[:], start=False,
                             stop=True)
            msg2_t = kp.tile([C, N], f32, tag="msg2")
            nc.vector.tensor_scalar(msg2_t[:], m2_ps[:], b["b_m2"][:], None,
                                    OP.add)
            # transpose, bounce through DRAM to re-tile as [4, 64, C]
            m2n_t = dp.tile([128, 2, C], bf, tag="m2n")
            for t in range(2):
                tp_ps = dps.tile([128, C], f32, tag="dps")
                nc.tensor.transpose(tp_ps[:], msg2_t[:, t * 128:(t + 1) * 128],
                                    idn_t[:, 0, 0:128])
                nc.scalar.activation(m2n_t[:, t], tp_ps[:], AF.Copy)
            nc.gpsimd.dma_start(
                m2bounce[:].rearrange("(t p) c -> p t c", p=128), m2n_t[:])
            m2n4_t = kp.tile([4, NQ, C], bf, tag="m2n4")
            nc.gpsimd.dma_start(
                m2n4_t[:], m2bounce[:].rearrange("(g q) c -> q g c", q=4))

            h1_ps = dps.tile([C, N], f32, tag="dps")
            nc.tensor.matmul(h1_ps[:], w["Wfe1a"][:], nodet_t[:], start=True,
                             stop=False)
            nc.tensor.matmul(h1_ps[:], w["Wfe1b"][:], hidt_t[:], start=False,
                             stop=True)
            hh1_t = dp.tile([C, N], f32, tag="hh1")
            nc.scalar.activation(hh1_t[:], h1_ps[:], AF.Relu, bias=b["b_fe1"][:])
            h2_ps = dps.tile([C, N], f32, tag="dps")
            nc.tensor.matmul(h2_ps[:], w["Wfe2"][:], hh1_t[:], start=True,
                             stop=True)
            ht_t = kp.tile([C, N], f32, tag="ht")
            nc.vector.tensor_scalar(ht_t[:], h2_ps[:], b["b_fe2"][:], None,
                                    OP.add)
            hn_t = kp.tile([128, 2, C], f32, tag="hn")
            for t in range(2):
                tp_ps = dps.tile([128, C], f32, tag="dps")
                nc.tensor.transpose(tp_ps[:], ht_t[:, t * 128:(t + 1) * 128],
                                    idn_t[:, 0, 0:128])
                nc.scalar.activation(hn_t[:, t], tp_ps[:], AF.Copy)

            ewcat_t = cp.tile([C, 384], bf, tag="ewcat")
            nc.vector.tensor_tensor(ewcat_t[:, 0:128], w["ew0"][:],
                                    w["ew2"][:], OP.subtract)
            nc.vector.tensor_copy(ewcat_t[:, 128:256], w["ew1"][:])
            nc.vector.tensor_scalar(ewcat_t[:, 256:384], w["ew2"][:], 2.0,
                                    None, OP.mult)

            # ============ main loop ============
            xq_tiles = [xp.tile([C, 512], bf, tag="xt%d" % g,
                                name="xt%d" % g) for g in range(NQ)]
            acc_t = kp.tile([C, 4 * JH], f32, tag="acc")
            nc.vector.memset(acc_t[:], -1e30)

            def do_edge(i):
                e_ps = eps_p.tile([JH, 384], f32, tag="eps", name="e_ps")
                nc.tensor.matmul(e_ps[:],
                                 xq_tiles[i // 4][:, (i % 4) * 128:
                                                  (i % 4) * 128 + 128],
                                 ewcat_t[:], start=True, stop=True)
                s1c = s1t_t[:, 0, i:i + 1]
                u_t = ep.tile([JH, C], f32, tag="u", name="u_t")
                nc.scalar.activation(u_t[:], e_ps[:, 256:384], AF.Copy,
                                     scale=s1c)
                v2_t = ep.tile([JH, C], f32, tag="v2", name="v2_t")
                nc.vector.scalar_tensor_tensor(v2_t[:], u_t[:], s1c,
                                               e_ps[:, 128:256], OP.mult,
                                               OP.add)
                o_t = ep.tile([JH, C], f32, tag="o", name="o_t")
                nc.vector.scalar_tensor_tensor(o_t[:], v2_t[:], s1c,
                                               e_ps[:, 0:128], OP.mult,
                                               OP.add)
                nc.sync.dma_start(edge_o[i], o_t[:])

            for g in range(NQ):
                nc.sync.dma_start_transpose(
                    xq_tiles[g][:], xh_d[g * 512:(g + 1) * 512, :])

            for g in range(NQ):
                xq = xq_tiles[g][:]
                pre_ps = mps.tile([C, 512], f32, tag="mps", name="pre_ps")
                nc.tensor.matmul(pre_ps[:], w["Wme"][:], xq, start=True,
                                 stop=False)
                nc.tensor.matmul(pre_ps[:], idb_t[:], m1g_t[:], start=False,
                                 stop=False)
                nc.tensor.matmul(pre_ps[:], m2n4_t[:, g, :], ind4_t[:],
                                 start=False, stop=True)
                h1q_t = mp.tile([C, 512], bf, tag="h1q", name="h1q_t")
                nc.scalar.activation(h1q_t[:], pre_ps[:], AF.Relu)
                p2_ps = mps.tile([C, 512], f32, tag="mps", name="p2_ps")
                nc.tensor.matmul(p2_ps[:], w["Wmlp1"][:], h1q_t[:], start=True,
                                 stop=True)
                h2q_t = mp.tile([C, 512], bf, tag="h2q", name="h2q_t")
                nc.scalar.activation(h2q_t[:], p2_ps[:], AF.Relu,
                                     bias=b["b_mlp1"][:])
                p3_ps = mps.tile([C, 512], f32, tag="mps", name="p3_ps")
                nc.tensor.matmul(p3_ps[:], w["Wmlp2"][:], h2q_t[:], start=True,
                                 stop=True)
                nc.vector.tensor_tensor(acc_t[:], acc_t[:], p3_ps[:], OP.max)

                if g >= EDGE_LAG:
                    gg = g - EDGE_LAG
                    for q in range(4):
                        do_edge(4 * gg + q)

            for gg in range(NQ - EDGE_LAG, NQ):
                for q in range(4):
                    do_edge(4 * gg + q)

            # ============ tail: msgs, cheb node side, out head, LN ============
            mx1_t = dp.tile([C, JH], f32, tag="mx1")
            nc.vector.tensor_tensor(mx1_t[:], acc_t[:, 0:128],
                                    acc_t[:, 128:256], OP.max)
            mx2_t = dp.tile([C, JH], f32, tag="mx2")
            nc.vector.tensor_tensor(mx2_t[:], acc_t[:, 256:384],
                                    acc_t[:, 384:512], OP.max)
            mx3_t = dp.tile([C, JH], f32, tag="mx3")
            nc.vector.tensor_tensor(mx3_t[:], mx1_t[:], mx2_t[:], OP.max)
            msgs_t = kp.tile([C, JH], f32, tag="msgs")
            nc.vector.tensor_scalar(msgs_t[:], mx3_t[:], b["b_mlp2"][:], None,
                                    OP.add)

            cn1_t = kp.tile([128, 2, C], f32, tag="cn1")
            for t in range(2):
                c_ps = dps.tile([128, C], f32, tag="dps")
                for a in range(2):
                    nc.tensor.matmul(c_ps[:], s1t_t[:, a, t * 128:(t + 1) * 128],
                                     hn_t[:, a], start=(a == 0), stop=(a == 1))
                nc.scalar.activation(cn1_t[:, t], c_ps[:], AF.Copy)
            c2_ps = dps.tile([128, C], f32, tag="dps")
            for a in range(2):
                nc.tensor.matmul(c2_ps[:], s1t_t[:, a, 0:128], cn1_t[:, a],
                                 start=(a == 0), stop=(a == 1))
            cn2_t = kp.tile([128, C], f32, tag="cn2")
            nc.vector.scalar_tensor_tensor(cn2_t[:], c2_ps[:], 2.0,
                                           hn_t[:, 0], OP.mult, OP.subtract)
            cn1T_ps = dps.tile([128, C], f32, tag="dps")
            nc.tensor.transpose(cn1T_ps[:], cn1_t[:, 0], idn_t[:, 0, 0:128])
            cn1T_t = dp.tile([C, JH], f32, tag="cn1T")
            nc.scalar.activation(cn1T_t[:], cn1T_ps[:], AF.Copy)
            cn2T_ps = dps.tile([128, C], f32, tag="dps")
            nc.tensor.transpose(cn2T_ps[:], cn2_t[:], idn_t[:, 0, 0:128])
            cn2T_t = dp.tile([C, JH], f32, tag="cn2T")
            nc.scalar.activation(cn2T_t[:], cn2T_ps[:], AF.Copy)

            no_t = dp.tile([C, JH], f32, tag="no")
            tmp_t = dp.tile([C, JH], f32, tag="tmp")
            for k, cnk in enumerate([ht_t[:, 0:JH], cn1T_t[:], cn2T_t[:]]):
                pj_ps = dps.tile([C, JH], f32, tag="dps")
                nc.tensor.matmul(pj_ps[:], w["nw%d" % k][:], msgs_t[:],
                                 start=True, stop=True)
                if k == 0:
                    nc.vector.tensor_tensor(no_t[:], cnk, pj_ps[:], OP.mult)
                else:
                    nc.vector.tensor_tensor(tmp_t[:], cnk, pj_ps[:], OP.mult)
                    nc.vector.tensor_tensor(no_t[:], no_t[:], tmp_t[:], OP.add)

            oh_ps = dps.tile([C, JH], f32, tag="dps")
            nc.tensor.matmul(oh_ps[:], w["Wo1"][:], msgs_t[:], start=True,
                             stop=False)
            nc.tensor.matmul(oh_ps[:], w["Wo2"][:], no_t[:], start=False,
                             stop=False)
            nc.tensor.matmul(oh_ps[:], w["Wo3a"][:], nodet_t[:, 0:JH],
                             start=False, stop=False)
            nc.tensor.matmul(oh_ps[:], w["Wo3b"][:], hidt_t[:, 0:JH],
                             start=False, stop=True)
            bo_t = dp.tile([C, 1], f32, tag="bo")
            nc.vector.tensor_tensor(bo_t[:], b["b_o1"][:], b["b_o2"][:], OP.add)
            nc.vector.tensor_tensor(bo_t[:], bo_t[:], b["b_o3"][:], OP.add)
            opre_t = kp.tile([C, JH], f32, tag="opre")
            nc.vector.tensor_scalar(opre_t[:], oh_ps[:], bo_t[:], None, OP.add)

            sq_t = dp.tile([C, JH], f32, tag="sq")
            nc.scalar.activation(sq_t[:], opre_t[:], AF.Square)
            mu_ps = dps.tile([1, JH], f32, tag="dps")
            nc.tensor.matmul(mu_ps[:], onesc_t[:], opre_t[:], start=True,
                             stop=True)
            s2_ps = dps.tile([1, JH], f32, tag="dps")
            nc.tensor.matmul(s2_ps[:], onesc_t[:], sq_t[:], start=True,
                             stop=True)
            mu_t = dp.tile([1, JH], f32, tag="mu")
            nc.vector.tensor_scalar(mu_t[:], mu_ps[:], 1.0 / C, None, OP.mult)
            musq_t = dp.tile([1, JH], f32, tag="musq")
            nc.scalar.activation(musq_t[:], mu_t[:], AF.Square)
            var_t = dp.tile([1, JH], f32, tag="var")
            nc.vector.tensor_scalar(var_t[:], s2_ps[:], 1.0 / C, None, OP.mult)
            nc.vector.tensor_tensor(var_t[:], var_t[:], musq_t[:], OP.subtract)
            nc.vector.tensor_scalar(var_t[:], var_t[:], float(LN_EPS), None,
                                    OP.add)
            sd_t = dp.tile([1, JH], f32, tag="sd")
            nc.scalar.activation(sd_t[:], var_t[:], AF.Sqrt)
            r_t = dp.tile([1, JH], f32, tag="r")
            nc.vector.reciprocal(r_t[:], sd_t[:])
            u2_t = dp.tile([1, JH], f32, tag="u2")
            nc.vector.tensor_tensor(u2_t[:], mu_t[:], r_t[:], OP.mult)
            one1 = dp.tile([1, 1], f32, tag="one1")
            nc.vector.memset(one1[:], 1.0)
            rc_ps = dps.tile([JH, 1], f32, tag="dps")
            nc.tensor.matmul(rc_ps[:], r_t[:], one1[:], start=True, stop=True)
            rc_t = dp.tile([JH, 1], f32, tag="rc")
            nc.scalar.activation(rc_t[:], rc_ps[:], AF.Copy)
            uc_ps = dps.tile([JH, 1], f32, tag="dps")
            nc.tensor.matmul(uc_ps[:], u2_t[:], one1[:], start=True, stop=True)
            uc_t = dp.tile([JH, 1], f32, tag="uc")
            nc.scalar.activation(uc_t[:], uc_ps[:], AF.Copy)
            on_ps = dps.tile([JH, C], f32, tag="dps")
            nc.tensor.transpose(on_ps[:], opre_t[:], idn_t[:, 0, 0:128])
            yn_t = dp.tile([JH, C], f32, tag="yn")
            nc.vector.tensor_scalar(yn_t[:], on_ps[:], rc_t[:], uc_t[:],
                                    OP.mult, OP.subtract)
            yt_ps = dps.tile([C, JH], f32, tag="dps")
            nc.tensor.transpose(yt_ps[:], yn_t[:], idn_t[:, 0, 0:128])
            yf_t = dp.tile([C, JH], f32, tag="yf")
            nc.vector.tensor_scalar(yf_t[:], yt_ps[:], b["ln_g"][:],
                                    b["ln_b"][:], OP.mult, OP.add)
            nc.sync.dma_start(out_o[:], yf_t[:])

    nc.compile()
    return nc


def _get_nc():
    if "nc" not in _CACHE:
        _CACHE["nc"] = _build_nc()
    return _CACHE["nc"]


def make_in_maps(inputs):
    """Build the 8 per-core input dicts (host-side sharding)."""
    node_fts = np.asarray(inputs["node_fts"], np.float32)
    edge_fts = np.asarray(inputs["edge_fts"], np.float32)
    graph_fts = np.asarray(inputs["graph_fts"], np.float32)
    hidden = np.asarray(inputs["hidden"], np.float32)
    adj = np.asarray(inputs["adj_matrix"], np.float32)

    wmap_c = {}
    wmap_c["Wm1a"] = inputs["W_m1"][0:C]
    wmap_c["Wm1b"] = inputs["W_m1"][C:2 * C]
    wmap_c["Wm2a"] = inputs["W_m2"][0:C]
    wmap_c["Wm2b"] = inputs["W_m2"][C:2 * C]
    wmap_c["Wme"] = inputs["W_me"]
    wmap_c["Wmg"] = inputs["W_mg"]
    wmap_c["Wfe1a"] = inputs["W_fe1"][0:C]
    wmap_c["Wfe1b"] = inputs["W_fe1"][C:2 * C]
    wmap_c["Wfe2"] = inputs["W_fe2"]
    wmap_c["Wmlp1"] = inputs["W_mlp1"]
    wmap_c["Wmlp2"] = inputs["W_mlp2"]
    wmap_c["Wo1"] = inputs["W_o1"]
    wmap_c["Wo2"] = inputs["W_o2"]
    wmap_c["Wo3a"] = inputs["W_o3"][0:C]
    wmap_c["Wo3b"] = inputs["W_o3"][C:2 * C]
    for k in range(3):
        wmap_c["nw%d" % k] = inputs["node_weights"][k]
        wmap_c["ew%d" % k] = inputs["edge_weights"][k]
    wmap = {}
    for n, a in wmap_c.items():
        a = np.ascontiguousarray(np.asarray(a, np.float32))
        if n in WB_NAMES:
            wmap[n] = a.astype(bf16)
        else:
            wmap[n] = a
    bmap = {
        "b_m1": inputs["b_m1"], "b_me": inputs["b_me"], "b_mg": inputs["b_mg"],
        "b_m2": inputs["b_m2"], "b_fe1": inputs["b_fe1"],
        "b_fe2": inputs["b_fe2"], "b_mlp1": inputs["b_mlp1"],
        "b_mlp2": inputs["b_mlp2"], "b_o1": inputs["b_o1"],
        "b_o2": inputs["b_o2"], "b_o3": inputs["b_o3"],
        "ln_g": inputs["ln_g"], "ln_b": inputs["ln_b"],
    }
    bmap = {n: np.ascontiguousarray(
        np.asarray(a, np.float32).reshape(C, 1)) for n, a in bmap.items()}

    in_maps = []
    perms = [np.arange(N), np.concatenate([np.arange(128, 256),
                                           np.arange(0, 128)])]
    for core in range(N_CORES):
        bidx, h = core // 2, core % 2
        perm = perms[h]
        xs = edge_fts[bidx][perm][:, perm[:JH], :]       # [N, JH, C]
        a2 = adj[bidx][np.ix_(perm, perm)]
        m = {
            "xh": np.ascontiguousarray(xs.reshape(N * JH, C).astype(bf16)),
            "adjt": np.ascontiguousarray(a2.T),
            "nodet": np.ascontiguousarray(node_fts[bidx][perm].T),
            "hidt": np.ascontiguousarray(hidden[bidx][perm].T),
            "graphc": np.ascontiguousarray(graph_fts[bidx].reshape(C, 1)),
        }
        m.update(wmap)
        m.update(bmap)
        in_maps.append(m)
    return in_maps


def gather(results):
    """Assemble full outputs from the 8 per-core result dicts."""
    out = np.zeros((B, N, C), np.float32)
    edge_out = np.zeros((B, N, N, C), np.float32)
    perms = [np.arange(N), np.concatenate([np.arange(128, 256),
                                           np.arange(0, 128)])]
    for core in range(N_CORES):
        bidx, h = core // 2, core % 2
        perm = perms[h]
        jcols = perm[:JH]
        edge_out[bidx][np.ix_(perm, jcols)] = results[core]["edge_o"]
        out[bidx][jcols] = results[core]["out_o"].T
    return out, edge_out


def kernel(**inputs):
    from concourse.bass_utils import run_bass_kernel_spmd

    nc = _get_nc()
    in_maps = make_in_maps(inputs)
    res = run_bass_kernel_spmd(nc, in_maps, list(range(N_CORES)))
    return gather(res.results)


# revision 13
# speedup vs baseline: 1.3597x; 1.1941x over previous
"""Trainium2 Bass kernel for nn_ChebyshevGraphConv (B=4, N=256, C=128, K=3).

Sharding: 8 cores = (batch b, node-half h). Core (b,h) owns node columns
jh = h*128:(h+1)*128 of the N x N edge grid for batch b:
  - msg side: msgs[b, jh, :] is complete locally (max over ALL i).
  - edge side: edge_out[b, :, jh, :] (pointwise in (i, j)).
  - phase D (laplacian/sigma/cheb + small per-batch compute) is done
    redundantly per core pair; each core writes out[b, jh, :].
No cross-core communication. One SPMD program with jh=0 baked in: cores
owning the second half receive block-permuted (node-relabeled) inputs,
un-permuted by the host on gather (the computation is equivariant under
node relabeling).

Device design:
  - X_T = edge_fts[b, :, jh, :]^T as [C, 32768] SBUF bf16, loaded with
    DMA-transpose straight from DRAM. All N^2-sized matmuls run in bf16
    (PE's fast fp32 path truncates the stationary operand to ~bf16
    anyway, so bf16 costs no extra accuracy); accumulation is fp32 in
    PSUM. All small per-batch math (laplacian, sigma, h, output head,
    layernorm) is plain fp32.
  - msg pipeline in A-layout [C, rows] 4-i quads (free=512).
  - edge pipeline in B-layout [j, c] per i with Horner combine
    out = s1*(s1*(X@2W2) + X@W1) + X@(W0-W2)  (s1 = cheb_lap col):
    ACT scale-copy + 2 DVE scalar_tensor_tensor. Edge work lags the
    quad loop by EDGE_LAG quads so the sigma chain is off the critical
    path.
  - sigma (largest singular value) via repeated squaring of M = L^T L
    with Frobenius normalization + Rayleigh quotient (the top spectrum
    of L is a near-continuum; plain power iteration cannot converge).
"""

import numpy as np
import ml_dtypes

B, N, C = 4, 256, 128
JH = 128
NQ = 64
EPS = 1e-5
LN_EPS = 1e-5
N_CORES = 8
N_SQUARINGS = 11
EDGE_LAG = 12

bf16 = ml_dtypes.bfloat16

_CACHE = {}

WF_NAMES = ["Wm1a", "Wm1b", "Wm2a", "Wm2b", "Wmg", "Wfe1a", "Wfe1b", "Wfe2",
            "Wo1", "Wo2", "Wo3a", "Wo3b", "nw0", "nw1", "nw2"]
WB_NAMES = ["Wme", "Wmlp1", "Wmlp2", "ew0", "ew1", "ew2"]
B_NAMES = ["b_m1", "b_me", "b_mg", "b_m2", "b_fe1", "b_fe2", "b_mlp1",
           "b_mlp2", "b_o1", "b_o2", "b_o3", "ln_g", "ln_b"]


def _build_nc():
    import concourse.bacc as bacc
    import concourse.mybir as mybir
    from concourse.tile import TileContext

    f32 = mybir.dt.float32
    bf = mybir.dt.bfloat16
    AF = mybir.ActivationFunctionType
    OP = mybir.AluOpType
    AX = mybir.AxisListType

    nc = bacc.Bacc("TRN2", target_bir_lowering=False)

    # ---------------- DRAM I/O ----------------
    xh_d = nc.dram_tensor("xh", [N * JH, C], bf, kind="ExternalInput")
    adjt_d = nc.dram_tensor("adjt", [N, N], f32, kind="ExternalInput")
    nodet_d = nc.dram_tensor("nodet", [C, N], f32, kind="ExternalInput")
    hidt_d = nc.dram_tensor("hidt", [C, N], f32, kind="ExternalInput")
    graph_d = nc.dram_tensor("graphc", [C, 1], f32, kind="ExternalInput")
    wd = {}
    for n in WF_NAMES:
        wd[n] = nc.dram_tensor(n, [C, C], f32, kind="ExternalInput")
    for n in WB_NAMES:
        wd[n] = nc.dram_tensor(n, [C, C], bf, kind="ExternalInput")
    bd = {n: nc.dram_tensor(n, [C, 1], f32, kind="ExternalInput") for n in B_NAMES}

    edge_o = nc.dram_tensor("edge_o", [N, JH, C], f32, kind="ExternalOutput")
    out_o = nc.dram_tensor("out_o", [C, JH], f32, kind="ExternalOutput")
    m2bounce = nc.dram_tensor("m2bounce", [N, C], bf)   # internal scratch

    id_cb = nc.inline_tensor(np.eye(C, dtype=bf16), "id_cb")
    eye256 = np.ascontiguousarray(
        np.eye(N, dtype=np.float32).reshape(2, 128, N).transpose(1, 0, 2))
    id_n = nc.inline_tensor(eye256, "id_n")
    ind4_c = nc.inline_tensor(
        np.kron(np.eye(4), np.ones((1, JH))).astype(bf16), "ind4")
    ones_col = nc.inline_tensor(np.ones((C, 1), np.float32), "ones_col")
    ones_row = nc.inline_tensor(np.ones((1, C), np.float32), "ones_row")

    with TileContext(nc) as tc:
        with tc.tile_pool(name="const", bufs=1) as cp, \
             tc.tile_pool(name="xt", bufs=1) as xp, \
             tc.tile_pool(name="msg", bufs=3) as mp, \
             tc.tile_pool(name="edge", bufs=4) as ep, \
             tc.tile_pool(name="small", bufs=1) as kp, \
             tc.tile_pool(name="pd", bufs=1) as dp, \
             tc.tile_pool(name="mps", bufs=3, space="PSUM") as mps, \
             tc.tile_pool(name="eps", bufs=3, space="PSUM") as eps_p, \
             tc.tile_pool(name="dps", bufs=2, space="PSUM") as dps:

            # ------------- loads (phase-D inputs first) -------------
            at_t = cp.tile([128, 2, N], f32, tag="at")
            nc.gpsimd.dma_start(
                at_t[:], adjt_d[:].rearrange("(t p) n -> p t n", p=128))
            idn_t = cp.tile([128, 2, N], f32, tag="idn")
            nc.gpsimd.dma_start(idn_t[:], id_n[:])
            onesc_t = cp.tile([C, 1], f32, tag="onesc")
            nc.gpsimd.dma_start(onesc_t[:], ones_col[:])
            onesr_t = cp.tile([1, C], f32, tag="onesr")
            nc.gpsimd.dma_start(onesr_t[:], ones_row[:])
            nodet_t = cp.tile([C, N], f32, tag="nodet")
            nc.gpsimd.dma_start(nodet_t[:], nodet_d[:])
            hidt_t = cp.tile([C, N], f32, tag="hidt")
            nc.gpsimd.dma_start(hidt_t[:], hidt_d[:])
            graph_t = cp.tile([C, 1], f32, tag="graph")
            nc.gpsimd.dma_start(graph_t[:], graph_d[:])
            w = {}
            for n in WB_NAMES:
                w[n] = cp.tile([C, C], bf, tag=n, name="w_" + n)
                nc.gpsimd.dma_start(w[n][:], wd[n][:])
            for n in WF_NAMES:
                w[n] = cp.tile([C, C], f32, tag=n, name="w_" + n)
                nc.gpsimd.dma_start(w[n][:], wd[n][:])
            b = {}
            for n in B_NAMES:
                b[n] = cp.tile([C, 1], f32, tag="t" + n, name="t" + n)
                nc.gpsimd.dma_start(b[n][:], bd[n][:])
            idb_t = cp.tile([C, C], bf, tag="idb")
            nc.gpsimd.dma_start(idb_t[:], id_cb[:])
            ind4_t = cp.tile([4, 4 * JH], bf, tag="ind4")
            nc.gpsimd.dma_start(ind4_t[:], ind4_c[:])

            # ============ PHASE D head: L, sigma, s1_T ============
            deg_t = kp.tile([128, 2], f32, tag="deg")
            nc.vector.tensor_reduce(deg_t[:], at_t[:], axis=AX.X, op=OP.add)
            sdeg_t = kp.tile([128, 2], f32, tag="sdeg")
            nc.scalar.activation(sdeg_t[:], deg_t[:], AF.Sqrt)
            dinv_t = kp.tile([128, 2], f32, tag="dinv")
            nc.vector.reciprocal(dinv_t[:], sdeg_t[:])
            dvr_ps = dps.tile([1, N], f32, tag="dps")
            for t in range(2):
                nc.tensor.matmul(dvr_ps[:, t * 128:(t + 1) * 128],
                                 dinv_t[:, t:t + 1],
                                 idn_t[:, t, t * 128:(t + 1) * 128],
                                 start=(t == 0), stop=(t == 1))
            dvr_t = kp.tile([1, N], f32, tag="dvr")
            nc.scalar.activation(dvr_t[:], dvr_ps[:], AF.Copy)
            lt_t = kp.tile([128, 2, N], f32, tag="lt")
            ln_t = kp.tile([128, 2, N], f32, tag="ln")
            for t in range(2):
                outer_ps = dps.tile([128, N], f32, tag="dps")
                nc.tensor.matmul(outer_ps[:], dvr_t[:, t * 128:(t + 1) * 128],
                                 dvr_t[:], start=True, stop=True)
                prod_t = dp.tile([128, N], f32, tag="prod")
                nc.vector.tensor_tensor(prod_t[:], at_t[:, t], outer_ps[:],
                                        OP.mult)
                nc.vector.tensor_tensor(lt_t[:, t], idn_t[:, t], prod_t[:],
                                        OP.subtract)
            for a in range(2):
                lnp = dps.tile([128, N], f32, tag="dps")
                for bb in range(2):
                    nc.tensor.transpose(lnp[:, bb * 128:(bb + 1) * 128],
                                        lt_t[:, bb, a * 128:(a + 1) * 128],
                                        idn_t[:, 0, 0:128])
                nc.scalar.activation(ln_t[:, a], lnp[:], AF.Copy)
            m0_t = kp.tile([128, 2, N], f32, tag="m0")
            mw_t = dp.tile([128, 2, N], f32, tag="mw", bufs=2)
            for t in range(2):
                mb_ps = dps.tile([128, N], f32, tag="dps")
                for a in range(2):
                    nc.tensor.matmul(mb_ps[:], ln_t[:, a, t * 128:(t + 1) * 128],
                                     ln_t[:, a], start=(a == 0), stop=(a == 1))
                nc.scalar.activation(m0_t[:, t], mb_ps[:], AF.Copy)
                nc.vector.tensor_copy(mw_t[:, t], mb_ps[:])
            for it in range(N_SQUARINGS):
                sqc_t = dp.tile([128, 2], f32, tag="sqc")
                for t in range(2):
                    scr = dp.tile([128, N], f32, tag="scr")
                    nc.scalar.activation(scr[:], mw_t[:, t], AF.Square,
                                         accum_out=sqc_t[:, t:t + 1])
                fro_ps = dps.tile([1, 1], f32, tag="dps")
                for t in range(2):
                    nc.tensor.matmul(fro_ps[:], sqc_t[:, t:t + 1], onesc_t[:],
                                     start=(t == 0), stop=(t == 1))
                fro_t = dp.tile([1, 1], f32, tag="fro")
                nc.scalar.activation(fro_t[:], fro_ps[:], AF.Sqrt)
                rfro_t = dp.tile([1, 1], f32, tag="rfro")
                nc.vector.reciprocal(rfro_t[:], fro_t[:])
                rfc_ps = dps.tile([C, 1], f32, tag="dps")
                nc.tensor.matmul(rfc_ps[:], onesr_t[:], rfro_t[:], start=True,
                                 stop=True)
                rfc_t = dp.tile([C, 1], f32, tag="rfc")
                nc.scalar.activation(rfc_t[:], rfc_ps[:], AF.Copy)
                mn_t = dp.tile([128, 2, N], f32, tag="mn", bufs=2)
                for t in range(2):
                    nc.vector.tensor_scalar(mn_t[:, t], mw_t[:, t], rfc_t[:],
                                            None, OP.mult)
                mw_t = dp.tile([128, 2, N], f32, tag="mw", bufs=2)
                for t in range(2):
                    sq_ps = dps.tile([128, N], f32, tag="dps")
                    for a in range(2):
                        nc.tensor.matmul(sq_ps[:],
                                         mn_t[:, a, t * 128:(t + 1) * 128],
                                         mn_t[:, a], start=(a == 0),
                                         stop=(a == 1))
                    nc.vector.tensor_copy(mw_t[:, t], sq_ps[:])
            v_t = kp.tile([128, 2], f32, tag="v")
            nc.vector.tensor_reduce(v_t[:], mw_t[:], axis=AX.X, op=OP.add)
            wv_t = kp.tile([128, 2], f32, tag="wv")
            for t in range(2):
                wv_ps = dps.tile([128, 1], f32, tag="dps")
                for a in range(2):
                    nc.tensor.matmul(wv_ps[:], m0_t[:, a, t * 128:(t + 1) * 128],
                                     v_t[:, a:a + 1], start=(a == 0),
                                     stop=(a == 1))
                nc.scalar.activation(wv_t[:, t:t + 1], wv_ps[:], AF.Copy)
            num_ps = dps.tile([1, 1], f32, tag="dps")
            den_ps = dps.tile([1, 1], f32, tag="dps")
            for t in range(2):
                nc.tensor.matmul(num_ps[:], v_t[:, t:t + 1], wv_t[:, t:t + 1],
                                 start=(t == 0), stop=(t == 1))
            for t in range(2):
                nc.tensor.matmul(den_ps[:], v_t[:, t:t + 1], v_t[:, t:t + 1],
                                 start=(t == 0), stop=(t == 1))
            den_s = dp.tile([1, 1], f32, tag="dens")
            nc.scalar.activation(den_s[:], den_ps[:], AF.Copy)
            rden_t = dp.tile([1, 1], f32, tag="rden")
            nc.vector.reciprocal(rden_t[:], den_s[:])
            lam_t = dp.tile([1, 1], f32, tag="lam")
            nc.vector.tensor_tensor(lam_t[:], num_ps[:], rden_t[:], OP.mult)
            sig_t = dp.tile([1, 1], f32, tag="sig")
            nc.scalar.activation(sig_t[:], lam_t[:], AF.Sqrt)
            sige_t = dp.tile([1, 1], f32, tag="sige")
            nc.vector.tensor_scalar(sige_t[:], sig_t[:], float(EPS), None,
                                    OP.add)
            rsig_t = dp.tile([1, 1], f32, tag="rsig")
            nc.vector.reciprocal(rsig_t[:], sige_t[:])
            c2_t = dp.tile([1, 1], f32, tag="c2")
            nc.vector.tensor_scalar(c2_t[:], rsig_t[:], 2.0, None, OP.mult)
            c2c_ps = dps.tile([C, 1], f32, tag="dps")
            nc.tensor.matmul(c2c_ps[:], onesr_t[:], c2_t[:], start=True,
                             stop=True)
            c2c_t = kp.tile([C, 1], f32, tag="c2c")
            nc.scalar.activation(c2c_t[:], c2c_ps[:], AF.Copy)
            s1t_t = kp.tile([128, 2, N], f32, tag="s1t")
            for t in range(2):
                nc.vector.scalar_tensor_tensor(s1t_t[:, t], lt_t[:, t],
                                               c2c_t[:], idn_t[:, t],
                                               OP.mult, OP.subtract)

            # ============ small precomputes: m1g, msg2, h, ewcat ============
            g_ps = dps.tile([C, 1], f32, tag="dps")
            nc.tensor.matmul(g_ps[:], w["Wmg"][:], graph_t[:], start=True,
                             stop=True)
            gcol_t = dp.tile([C, 1], f32, tag="gcol")
            nc.scalar.activation(gcol_t[:], g_ps[:], AF.Copy)
            gb_t = kp.tile([C, 1], f32, tag="gb")
            nc.vector.tensor_tensor(gb_t[:], gcol_t[:], b["b_m1"][:], OP.add)
            nc.vector.tensor_tensor(gb_t[:], gb_t[:], b["b_me"][:], OP.add)
            nc.vector.tensor_tensor(gb_t[:], gb_t[:], b["b_mg"][:], OP.add)

            m1_ps = dps.tile([C, JH], f32, tag="dps")
            nc.tensor.matmul(m1_ps[:], w["Wm1a"][:], nodet_t[:, 0:JH],
                             start=True, stop=False)
            nc.tensor.matmul(m1_ps[:], w["Wm1b"][:], hidt_t[:, 0:JH],
                             start=False, stop=True)
            m1g_t = kp.tile([C, 4 * JH], bf, tag="m1g")
            for q in range(4):
                nc.vector.tensor_scalar(m1g_t[:, q * JH:(q + 1) * JH],
                                        m1_ps[:], gb_t[:], None, OP.add)

            m2_ps = dps.tile([C, N], f32, tag="dps")
            nc.tensor.matmul(m2_ps[:], w["Wm2a"][:], nodet_t[:], start=True,
                             stop=False)
            nc.tensor.matmul(m2_ps[:], w["Wm2b"][:], hidt_t[:], start=False,
                             stop=True)
            msg2_t = kp.tile([C, N], f32, tag="msg2")
            nc.vector.tensor_scalar(msg2_t[:], m2_ps[:], b["b_m2"][:], None,
                                    OP.add)
            # transpose, bounce through DRAM to re-tile as [4, 64, C]
            m2n_t = dp.tile([128, 2, C], bf, tag="m2n")
            for t in range(2):
                tp_ps = dps.tile([128, C], f32, tag="dps")
                nc.tensor.transpose(tp_ps[:], msg2_t[:, t * 128:(t + 1) * 128],
                                    idn_t[:, 0, 0:128])
                nc.scalar.activation(m2n_t[:, t], tp_ps[:], AF.Copy)
            nc.gpsimd.dma_start(
                m2bounce[:].rearrange("(t p) c -> p t c", p=128), m2n_t[:])
            m2n4_t = kp.tile([4, NQ, C], bf, tag="m2n4")
            nc.gpsimd.dma_start(
                m2n4_t[:], m2bounce[:].rearrange("(g q) c -> q g c", q=4))

            h1_ps = dps.tile([C, N], f32, tag="dps")
            nc.tensor.matmul(h1_ps[:], w["Wfe1a"][:], nodet_t[:], start=True,
                             stop=False)
            nc.tensor.matmul(h1_ps[:], w["Wfe1b"][:], hidt_t[:], start=False,
                             stop=True)
            hh1_t = dp.tile([C, N], f32, tag="hh1")
            nc.scalar.activation(hh1_t[:], h1_ps[:], AF.Relu, bias=b["b_fe1"][:])
            h2_ps = dps.tile([C, N], f32, tag="dps")
            nc.tensor.matmul(h2_ps[:], w["Wfe2"][:], hh1_t[:], start=True,
                             stop=True)
            ht_t = kp.tile([C, N], f32, tag="ht")
            nc.vector.tensor_scalar(ht_t[:], h2_ps[:], b["b_fe2"][:], None,
                                    OP.add)
            hn_t = kp.tile([128, 2, C], f32, tag="hn")
            for t in range(2):
                tp_ps = dps.tile([128, C], f32, tag="dps")
                nc.tensor.transpose(tp_ps[:], ht_t[:, t * 128:(t + 1) * 128],
                                    idn_t[:, 0, 0:128])
                nc.scalar.activation(hn_t[:, t], tp_ps[:], AF.Copy)

            ewcat_t = cp.tile([C, 384], bf, tag="ewcat")
            nc.vector.tensor_tensor(ewcat_t[:, 0:128], w["ew0"][:],
                                    w["ew2"][:], OP.subtract)
            nc.vector.tensor_copy(ewcat_t[:, 128:256], w["ew1"][:])
            nc.vector.tensor_scalar(ewcat_t[:, 256:384], w["ew2"][:], 2.0,
                                    None, OP.mult)

            # ============ main loop ============
            xq_tiles = [xp.tile([C, 512], bf, tag="xt%d" % g,
                                name="xt%d" % g) for g in range(NQ)]
            acc_t = kp.tile([C, 4 * JH], f32, tag="acc")
            nc.vector.memset(acc_t[:], -1e30)

            def do_edge(i):
                e_ps = eps_p.tile([JH, 384], f32, tag="eps", name="e_ps")
                nc.tensor.matmul(e_ps[:],
                                 xq_tiles[i // 4][:, (i % 4) * 128:
                                                  (i % 4) * 128 + 128],
                                 ewcat_t[:], start=True, stop=True)
                s1c = s1t_t[:, 0, i:i + 1]
                u_t = ep.tile([JH, C], f32, tag="u", name="u_t")
                nc.scalar.activation(u_t[:], e_ps[:, 256:384], AF.Copy,
                                     scale=s1c)
                v2_t = ep.tile([JH, C], f32, tag="v2", name="v2_t")
                nc.vector.scalar_tensor_tensor(v2_t[:], u_t[:], s1c,
                                               e_ps[:, 128:256], OP.mult,
                                               OP.add)
                o_t = ep.tile([JH, C], f32, tag="o", name="o_t")
                nc.vector.scalar_tensor_tensor(o_t[:], v2_t[:], s1c,
                                               e_ps[:, 0:128], OP.mult,
                                               OP.add)
                nc.sync.dma_start(edge_o[i], o_t[:])

            for g in range(NQ):
                xn_t = mp.tile([128, 4, C], bf, tag="xn", name="xn_t")
                nc.sync.dma_start(
                    xn_t[:], xh_d[g * 512:(g + 1) * 512, :].rearrange(
                        "(a p) c -> p a c", p=128))
                xtp_ps = mps.tile([C, 512], bf, tag="mps", name="xtp_ps")
                for t in range(4):
                    nc.tensor.transpose(xtp_ps[:, t * 128:(t + 1) * 128],
                                        xn_t[:, t, :], idb_t[:])
                if g % 2 == 0:
                    nc.scalar.activation(xq_tiles[g][:], xtp_ps[:], AF.Copy)
                else:
                    nc.vector.tensor_copy(xq_tiles[g][:], xtp_ps[:])
                xq = xq_tiles[g][:]
                pre_ps = mps.tile([C, 512], f32, tag="mps", name="pre_ps")
                nc.tensor.matmul(pre_ps[:], w["Wme"][:], xq, start=True,
                                 stop=False)
                nc.tensor.matmul(pre_ps[:], idb_t[:], m1g_t[:], start=False,
                                 stop=False)
                nc.tensor.matmul(pre_ps[:], m2n4_t[:, g, :], ind4_t[:],
                                 start=False, stop=True)
                h1q_t = mp.tile([C, 512], bf, tag="h1q", name="h1q_t")
                nc.scalar.activation(h1q_t[:], pre_ps[:], AF.Relu)
                p2_ps = mps.tile([C, 512], f32, tag="mps", name="p2_ps")
                nc.tensor.matmul(p2_ps[:], w["Wmlp1"][:], h1q_t[:], start=True,
                                 stop=True)
                h2q_t = mp.tile([C, 512], bf, tag="h2q", name="h2q_t")
                nc.scalar.activation(h2q_t[:], p2_ps[:], AF.Relu,
                                     bias=b["b_mlp1"][:])
                p3_ps = mps.tile([C, 512], f32, tag="mps", name="p3_ps")
                nc.tensor.matmul(p3_ps[:], w["Wmlp2"][:], h2q_t[:], start=True,
                                 stop=True)
                nc.vector.tensor_tensor(acc_t[:], acc_t[:], p3_ps[:], OP.max)

                if g >= EDGE_LAG:
                    gg = g - EDGE_LAG
                    for q in range(4):
                        do_edge(4 * gg + q)

            for gg in range(NQ - EDGE_LAG, NQ):
                for q in range(4):
                    do_edge(4 * gg + q)

            # ============ tail: msgs, cheb node side, out head, LN ============
            mx1_t = dp.tile([C, JH], f32, tag="mx1")
            nc.vector.tensor_tensor(mx1_t[:], acc_t[:, 0:128],
                                    acc_t[:, 128:256], OP.max)
            mx2_t = dp.tile([C, JH], f32, tag="mx2")
            nc.vector.tensor_tensor(mx2_t[:], acc_t[:, 256:384],
                                    acc_t[:, 384:512], OP.max)
            mx3_t = dp.tile([C, JH], f32, tag="mx3")
            nc.vector.tensor_tensor(mx3_t[:], mx1_t[:], mx2_t[:], OP.max)
            msgs_t = kp.tile([C, JH], f32, tag="msgs")
            nc.vector.tensor_scalar(msgs_t[:], mx3_t[:], b["b_mlp2"][:], None,
                                    OP.add)

            cn1_t = kp.tile([128, 2, C], f32, tag="cn1")
            for t in range(2):
                c_ps = dps.tile([128, C], f32, tag="dps")
                for a in range(2):
                    nc.tensor.matmul(c_ps[:], s1t_t[:, a, t * 128:(t + 1) * 128],
                                     hn_t[:, a], start=(a == 0), stop=(a == 1))
                nc.scalar.activation(cn1_t[:, t], c_ps[:], AF.Copy)
            c2_ps = dps.tile([128, C], f32, tag="dps")
            for a in range(2):
                nc.tensor.matmul(c2_ps[:], s1t_t[:, a, 0:128], cn1_t[:, a],
                                 start=(a == 0), stop=(a == 1))
            cn2_t = kp.tile([128, C], f32, tag="cn2")
            nc.vector.scalar_tensor_tensor(cn2_t[:], c2_ps[:], 2.0,
                                           hn_t[:, 0], OP.mult, OP.subtract)
            cn1T_ps = dps.tile([128, C], f32, tag="dps")
            nc.tensor.transpose(cn1T_ps[:], cn1_t[:, 0], idn_t[:, 0, 0:128])
            cn1T_t = dp.tile([C, JH], f32, tag="cn1T")
            nc.scalar.activation(cn1T_t[:], cn1T_ps[:], AF.Copy)
            cn2T_ps = dps.tile([128, C], f32, tag="dps")
            nc.tensor.transpose(cn2T_ps[:], cn2_t[:], idn_t[:, 0, 0:128])
            cn2T_t = dp.tile([C, JH], f32, tag="cn2T")
            nc.scalar.activation(cn2T_t[:], cn2T_ps[:], AF.Copy)

            no_t = dp.tile([C, JH], f32, tag="no")
            tmp_t = dp.tile([C, JH], f32, tag="tmp")
            for k, cnk in enumerate([ht_t[:, 0:JH], cn1T_t[:], cn2T_t[:]]):
                pj_ps = dps.tile([C, JH], f32, tag="dps")
                nc.tensor.matmul(pj_ps[:], w["nw%d" % k][:], msgs_t[:],
                                 start=True, stop=True)
                if k == 0:
                    nc.vector.tensor_tensor(no_t[:], cnk, pj_ps[:], OP.mult)
                else:
                    nc.vector.tensor_tensor(tmp_t[:], cnk, pj_ps[:], OP.mult)
                    nc.vector.tensor_tensor(no_t[:], no_t[:], tmp_t[:], OP.add)

            oh_ps = dps.tile([C, JH], f32, tag="dps")
            nc.tensor.matmul(oh_ps[:], w["Wo1"][:], msgs_t[:], start=True,
                             stop=False)
            nc.tensor.matmul(oh_ps[:], w["Wo2"][:], no_t[:], start=False,
                             stop=False)
            nc.tensor.matmul(oh_ps[:], w["Wo3a"][:], nodet_t[:, 0:JH],
                             start=False, stop=False)
            nc.tensor.matmul(oh_ps[:], w["Wo3b"][:], hidt_t[:, 0:JH],
                             start=False, stop=True)
            bo_t = dp.tile([C, 1], f32, tag="bo")
            nc.vector.tensor_tensor(bo_t[:], b["b_o1"][:], b["b_o2"][:], OP.add)
            nc.vector.tensor_tensor(bo_t[:], bo_t[:], b["b_o3"][:], OP.add)
            opre_t = kp.tile([C, JH], f32, tag="opre")
            nc.vector.tensor_scalar(opre_t[:], oh_ps[:], bo_t[:], None, OP.add)

            sq_t = dp.tile([C, JH], f32, tag="sq")
            nc.scalar.activation(sq_t[:], opre_t[:], AF.Square)
            mu_ps = dps.tile([1, JH], f32, tag="dps")
            nc.tensor.matmul(mu_ps[:], onesc_t[:], opre_t[:], start=True,
                             stop=True)
            s2_ps = dps.tile([1, JH], f32, tag="dps")
            nc.tensor.matmul(s2_ps[:], onesc_t[:], sq_t[:], start=True,
                             stop=True)
            mu_t = dp.tile([1, JH], f32, tag="mu")
            nc.vector.tensor_scalar(mu_t[:], mu_ps[:], 1.0 / C, None, OP.mult)
            musq_t = dp.tile([1, JH], f32, tag="musq")
            nc.scalar.activation(musq_t[:], mu_t[:], AF.Square)
            var_t = dp.tile([1, JH], f32, tag="var")
            nc.vector.tensor_scalar(var_t[:], s2_ps[:], 1.0 / C, None, OP.mult)
            nc.vector.tensor_tensor(var_t[:], var_t[:], musq_t[:], OP.subtract)
            nc.vector.tensor_scalar(var_t[:], var_t[:], float(LN_EPS), None,
                                    OP.add)
            sd_t = dp.tile([1, JH], f32, tag="sd")
            nc.scalar.activation(sd_t[:], var_t[:], AF.Sqrt)
            r_t = dp.tile([1, JH], f32, tag="r")
            nc.vector.reciprocal(r_t[:], sd_t[:])
            u2_t = dp.tile([1, JH], f32, tag="u2")
            nc.vector.tensor_tensor(u2_t[:], mu_t[:], r_t[:], OP.mult)
            one1 = dp.tile([1, 1], f32, tag="one1")
            nc.vector.memset(one1[:], 1.0)
            rc_ps = dps.tile([JH, 1], f32, tag="dps")
            nc.tensor.matmul(rc_ps[:], r_t[:], one1[:], start=True, stop=True)
            rc_t = dp.tile([JH, 1], f32, tag="rc")
            nc.scalar.activation(rc_t[:], rc_ps[:], AF.Copy)
            uc_ps = dps.tile([JH, 1], f32, tag="dps")
            nc.tensor.matmul(uc_ps[:], u2_t[:], one1[:], start=True, stop=True)
            uc_t = dp.tile([JH, 1], f32, tag="uc")
            nc.scalar.activation(uc_t[:], uc_ps[:], AF.Copy)
            on_ps = dps.tile([JH, C], f32, tag="dps")
            nc.tensor.transpose(on_ps[:], opre_t[:], idn_t[:, 0, 0:128])
            yn_t = dp.tile([JH, C], f32, tag="yn")
            nc.vector.tensor_scalar(yn_t[:], on_ps[:], rc_t[:], uc_t[:],
                                    OP.mult, OP.subtract)
            yt_ps = dps.tile([C, JH], f32, tag="dps")
            nc.tensor.transpose(yt_ps[:], yn_t[:], idn_t[:, 0, 0:128])
            yf_t = dp.tile([C, JH], f32, tag="yf")
            nc.vector.tensor_scalar(yf_t[:], yt_ps[:], b["ln_g"][:],
                                    b["ln_b"][:], OP.mult, OP.add)
            nc.sync.dma_start(out_o[:], yf_t[:])

    nc.compile()
    return nc


def _get_nc():
    if "nc" not in _CACHE:
        _CACHE["nc"] = _build_nc()
    return _CACHE["nc"]


def make_in_maps(inputs):
    """Build the 8 per-core input dicts (host-side sharding)."""
    node_fts = np.asarray(inputs["node_fts"], np.float32)
    edge_fts = np.asarray(inputs["edge_fts"], np.float32)
    graph_fts = np.asarray(inputs["graph_fts"], np.float32)
    hidden = np.asarray(inputs["hidden"], np.float32)
    adj = np.asarray(inputs["adj_matrix"], np.float32)

    wmap_c = {}
    wmap_c["Wm1a"] = inputs["W_m1"][0:C]
    wmap_c["Wm1b"] = inputs["W_m1"][C:2 * C]
    wmap_c["Wm2a"] = inputs["W_m2"][0:C]
    wmap_c["Wm2b"] = inputs["W_m2"][C:2 * C]
    wmap_c["Wme"] = inputs["W_me"]
    wmap_c["Wmg"] = inputs["W_mg"]
    wmap_c["Wfe1a"] = inputs["W_fe1"][0:C]
    wmap_c["Wfe1b"] = inputs["W_fe1"][C:2 * C]
    wmap_c["Wfe2"] = inputs["W_fe2"]
    wmap_c["Wmlp1"] = inputs["W_mlp1"]
    wmap_c["Wmlp2"] = inputs["W_mlp2"]
    wmap_c["Wo1"] = inputs["W_o1"]
    wmap_c["Wo2"] = inputs["W_o2"]
    wmap_c["Wo3a"] = inputs["W_o3"][0:C]
    wmap_c["Wo3b"] = inputs["W_o3"][C:2 * C]
    for k in range(3):
        wmap_c["nw%d" % k] = inputs["node_weights"][k]
        wmap_c["ew%d" % k] = inputs["edge_weights"][k]
    wmap = {}
    for n, a in wmap_c.items():
        a = np.ascontiguousarray(np.asarray(a, np.float32))
        if n in WB_NAMES:
            wmap[n] = a.astype(bf16)
        else:
            wmap[n] = a
    bmap = {
        "b_m1": inputs["b_m1"], "b_me": inputs["b_me"], "b_mg": inputs["b_mg"],
        "b_m2": inputs["b_m2"], "b_fe1": inputs["b_fe1"],
        "b_fe2": inputs["b_fe2"], "b_mlp1": inputs["b_mlp1"],
        "b_mlp2": inputs["b_mlp2"], "b_o1": inputs["b_o1"],
        "b_o2": inputs["b_o2"], "b_o3": inputs["b_o3"],
        "ln_g": inputs["ln_g"], "ln_b": inputs["ln_b"],
    }
    bmap = {n: np.ascontiguousarray(
        np.asarray(a, np.float32).reshape(C, 1)) for n, a in bmap.items()}

    in_maps = []
    perms = [np.arange(N), np.concatenate([np.arange(128, 256),
                                           np.arange(0, 128)])]
    for core in range(N_CORES):
        bidx, h = core // 2, core % 2
        perm = perms[h]
        xs = edge_fts[bidx][perm][:, perm[:JH], :]       # [N, JH, C]
        a2 = adj[bidx][np.ix_(perm, perm)]
        m = {
            "xh": np.ascontiguousarray(xs.reshape(N * JH, C).astype(bf16)),
            "adjt": np.ascontiguousarray(a2.T),
            "nodet": np.ascontiguousarray(node_fts[bidx][perm].T),
            "hidt": np.ascontiguousarray(hidden[bidx][perm].T),
            "graphc": np.ascontiguousarray(graph_fts[bidx].reshape(C, 1)),
        }
        m.update(wmap)
        m.update(bmap)
        in_maps.append(m)
    return in_maps


def gather(results):
    """Assemble full outputs from the 8 per-core result dicts."""
    out = np.zeros((B, N, C), np.float32)
    edge_out = np.zeros((B, N, N, C), np.float32)
    perms = [np.arange(N), np.concatenate([np.arange(128, 256),
                                           np.arange(0, 128)])]
    for core in range(N_CORES):
        bidx, h = core // 2, core % 2
        perm = perms[h]
        jcols = perm[:JH]
        edge_out[bidx][np.ix_(perm, jcols)] = results[core]["edge_o"]
        out[bidx][jcols] = results[core]["out_o"].T
    return out, edge_out


def kernel(**inputs):
    from concourse.bass_utils import run_bass_kernel_spmd

    nc = _get_nc()
    in_maps = make_in_maps(inputs)
    res = run_bass_kernel_spmd(nc, in_maps, list(range(N_CORES)))
    return gather(res.results)
